# revision 10
# baseline (speedup 1.0000x reference)
"""Two-layer GCN (MultiOrderGraphLayer) Bass kernel for 8 Trainium2 cores.

Math: out = 0.5*(relu(A_hat@x@W1+b1) + relu(A_hat@x@W2+b2)) with
A_hat = D^-1/2 (A+I) D^-1/2.  Both layers share A_hat, so g = A_hat @ x is
computed once; the two small 128x128 matmuls run afterwards.

Normalization is split: norm_e = dinv[src]*dinv[dst] factors, so
  - host prescales x' = dinv (.) x  (bf16, one rounding),
  - the per-edge one-hot is a PURE 0/1 pattern (exact in fp8e4m3,
    precomputed on host, streamed from DRAM -- no DVE work),
  - dinv[dst] is applied in phase 2 as a per-partition ACT scale on the
    relu (nodes are partitions there); the bias matmul uses a
    sqrt(deg) row instead of ones so relu(dinv*(gW + sqrt(deg)*b)) =
    relu(dinv*gW + b) exactly.

Device algorithm (per core, feature-major g_T = [128 feat, nodes]):
  - nodes sharded 8 ways by row; edges partitioned by destination core.
  - self-loops appended as ordinary edges; edges grouped per 128-node
    output window, padded to 128-edge blocks (pad slots have all-zero
    one-hot columns, so gathered junk contributes nothing).
  - per block: dma_gather 128 rows of x' (256B bf16 each) and matmul
    t_T += xg^T @ S into PSUM over the window's blocks, S streamed fp8.
  - dma_gather indices are int16, so sources split into lo (<32768) and
    hi (>=32768) streams; each window accumulates in two phases.
  - finish: out[n, fo] = relu(dinv[n]*(g_T^T @ (0.5*W) + sqrt(deg)*0.5*b))
    summed over layers, written node-major straight from PSUM matmuls.
"""

import math
import numpy as np

N_NODES = 50000
D = 128
N_CORES = 8
SPLIT = 32768  # int16 gather index limit
WIN = 128      # output-window size in nodes (one-hot width / psum free dim)
CHUNK = 4096   # indices per dma_gather instruction (multiple of 128)
N_QUEUES = 4   # SWDGE queues; rotating queue_num 4x's gather bandwidth
OHG = 32       # one-hot blocks per DRAM slab load


# ---------------------------------------------------------------- host prep

def host_prep(edge_index, n_nodes, n_cores, split=SPLIT, chunk=CHUNK):
    """Preprocessing: edge partitioning by destination, window grouping,
    lo/hi source split, padding, 0/1 one-hot pattern + per-node dinv.

    Returns (meta, per_core_inputs) where per_core_inputs[c] is a dict of
    numpy arrays for core c's DRAM parameters (excluding x/W/b).
    """
    import ml_dtypes

    src = np.asarray(edge_index[0], dtype=np.int64)
    dst = np.asarray(edge_index[1], dtype=np.int64)
    deg = np.bincount(dst, minlength=n_nodes).astype(np.int64) + 1
    dinv_node = (1.0 / np.sqrt(deg)).astype(np.float32)
    sqdeg_node = np.sqrt(deg).astype(np.float32)

    loop = np.arange(n_nodes, dtype=np.int64)
    s_all = np.concatenate([src, loop])
    d_all = np.concatenate([dst, loop])

    npc = n_nodes // n_cores
    assert npc * n_cores == n_nodes
    nwin = math.ceil(npc / WIN)
    n_halves = 2 if n_nodes > split else 1

    per_core_sorted = []
    counts = np.zeros((n_cores, n_halves, nwin), np.int64)
    for c in range(n_cores):
        n0 = c * npc
        m = (d_all >= n0) & (d_all < n0 + npc)
        s, d = s_all[m], d_all[m]
        w = (d - n0) // WIN
        half = (s >= split).astype(np.int64) if n_halves == 2 else np.zeros_like(s)
        key = half * nwin + w
        order = np.argsort(key, kind="stable")
        s, d, key = s[order], d[order], key[order]
        cnt = np.bincount(key, minlength=n_halves * nwin)
        counts[c] = cnt.reshape(n_halves, nwin)
        per_core_sorted.append((s, d, cnt))

    # shared block structure: blocks per (half, window), equal across cores
    nblk = np.maximum(1, -(-counts.max(axis=0) // 128))  # [n_halves, nwin]
    half_tot = nblk.sum(axis=1) * 128                    # edge slots per half
    nblk_tot = int(nblk.sum())

    # chunk split per half (shared across cores); small warmup chunks so
    # the first PSUM windows start quickly
    chunk_sizes = []
    for h in range(n_halves):
        rem, sizes = int(half_tot[h]), []
        for warm in (1024, 1024, 2048, 2048):
            L = min(warm, rem)
            if L > 0:
                sizes.append(L)
                rem -= L
        while rem > 0:
            L = min(chunk, rem)
            sizes.append(L)
            rem -= L
        chunk_sizes.append(sizes)

    per_core_inputs = []
    for c in range(n_cores):
        s, d, cnt = per_core_sorted[c]
        offs = np.concatenate([[0], np.cumsum(cnt)])
        idx_h = [[] for _ in range(n_halves)]
        dl_parts = []
        for h in range(n_halves):
            for wi in range(nwin):
                k = h * nwin + wi
                a, b = int(offs[k]), int(offs[k + 1])
                L = int(nblk[h, wi]) * 128
                pad = L - (b - a)
                gs = np.concatenate([s[a:b] - h * split,
                                     np.zeros(pad, np.int64)])
                gd = np.concatenate([(d[a:b] - c * npc - wi * WIN).astype(np.int64),
                                     np.full(pad, -1, np.int64)])
                idx_h[h].append(gs.astype(np.int16))
                dl_parts.append(gd)

        # 0/1 one-hot pattern, fp8e4m3 exact: [128 lanes, nblk_tot*128]
        dl_stream = np.concatenate(dl_parts).reshape(-1, 128)  # [blk, lane]
        ohp = np.zeros((128, nblk_tot, 128), dtype=ml_dtypes.float8_e4m3)
        blk_i, lane_i = np.nonzero(dl_stream >= 0)
        ohp[lane_i, blk_i, dl_stream[blk_i, lane_i]] = 1.0
        # per-window dest dinv [128, nwin] and sqrt(deg) row [1, npc]
        dv = np.ones((128, nwin), np.float32)
        for wi in range(nwin):
            wl = min(WIN, npc - wi * WIN)
            dv[:wl, wi] = dinv_node[c * npc + wi * WIN:c * npc + wi * WIN + wl]
        core_in = {
            "ohpat": np.ascontiguousarray(ohp.reshape(128, nblk_tot * 128)),
            "dinvp": dv,
            "sqdeg": np.ascontiguousarray(
                sqdeg_node[c * npc:(c + 1) * npc].astype(ml_dtypes.bfloat16)
            ).reshape(1, npc),
        }
        # gather indices: wrapped [16, L/16] per chunk, replicated to 128 rows
        for h in range(n_halves):
            stream = np.concatenate(idx_h[h])
            cols, off = [], 0
            for L in chunk_sizes[h]:
                a = stream[off:off + L].reshape(-1, 16).T  # [16, L/16]
                cols.append(a)
                off += L
            wrapped = np.concatenate(cols, axis=1)         # [16, half_tot/16]
            core_in["idx_h%d" % h] = np.ascontiguousarray(
                np.tile(wrapped, (8, 1)))
        per_core_inputs.append(core_in)

    meta = dict(n_nodes=n_nodes, n_cores=n_cores, npc=npc, nwin=nwin,
                n_halves=n_halves, split=split, nblk=nblk,
                half_tot=half_tot, nblk_tot=nblk_tot, chunk=chunk,
                chunk_sizes=chunk_sizes, dinv_node=dinv_node)
    return meta, per_core_inputs


# ------------------------------------------------------------- bass program

def build_program(meta):
    import concourse.bacc as bacc
    import concourse.mybir as mybir
    import concourse.tile as tile
    from concourse import library_config

    f32 = mybir.dt.float32
    bf16 = mybir.dt.bfloat16
    fp8 = mybir.dt.float8e4
    i16 = mybir.dt.int16
    AF = mybir.ActivationFunctionType
    OP = mybir.AluOpType

    n_nodes = meta["n_nodes"]
    npc, nwin = meta["npc"], meta["nwin"]
    n_halves, split = meta["n_halves"], meta["split"]
    nblk, nblk_tot = meta["nblk"], meta["nblk_tot"]
    chunk = meta["chunk"]
    chunk_sizes = meta["chunk_sizes"]

    nc = bacc.Bacc("TRN2", num_swdge_queues=N_QUEUES)

    x_d = nc.declare_dram_parameter("x", [n_nodes, D], bf16, isOutput=False)
    oh_d = nc.declare_dram_parameter("ohpat", [128, nblk_tot * 128], fp8,
                                     isOutput=False)
    dv_d = nc.declare_dram_parameter("dinvp", [128, nwin], f32, isOutput=False)
    sq_d = nc.declare_dram_parameter("sqdeg", [1, npc], bf16, isOutput=False)
    idx_d = [nc.declare_dram_parameter("idx_h%d" % h,
                                       [128, int(meta["half_tot"][h]) // 16],
                                       i16, isOutput=False)
             for h in range(n_halves)]
    w1_d = nc.declare_dram_parameter("W1", [D, D], f32, isOutput=False)
    w2_d = nc.declare_dram_parameter("W2", [D, D], f32, isOutput=False)
    b1_d = nc.declare_dram_parameter("b1", [1, D], f32, isOutput=False)
    b2_d = nc.declare_dram_parameter("b2", [1, D], f32, isOutput=False)
    out_d = nc.declare_dram_parameter("out", [npc, D], f32, isOutput=True)

    WG = 4  # windows per phase-2 batch (one 512-wide psum bank)

    with tile.TileContext(nc) as tc:
        with (
            tc.tile_pool(name="const", bufs=1) as constp,
            tc.tile_pool(name="xg", bufs=6) as xgp,
            tc.tile_pool(name="oh", bufs=4) as ohp,
            tc.tile_pool(name="ps1", bufs=3, space="PSUM") as ps1,
            tc.tile_pool(name="ps2", bufs=2, space="PSUM") as ps2,
            tc.tile_pool(name="fin", bufs=3) as finp,
        ):
            # Q7 library holding DMAGatherAnt; must precede all gathers
            nc.gpsimd.load_library(library_config.mlp)

            # --- constants / metadata
            wts = {}
            for nm, src_d in (("w1", w1_d), ("w2", w2_d)):
                raw = constp.tile([128, 128], f32, tag=nm + "raw")
                nc.sync.dma_start(raw[:], src_d[:])
                half = constp.tile([128, 128], bf16, tag=nm + "half")
                nc.scalar.activation(half[:], raw[:], AF.Copy, scale=0.5)
                wts[nm] = half
            bias = {}
            for nm, src_d in (("b1", b1_d), ("b2", b2_d)):
                raw = constp.tile([1, 128], f32, tag=nm + "raw")
                nc.sync.dma_start(raw[:], src_d[:])
                half = constp.tile([1, 128], bf16, tag=nm + "half")
                nc.scalar.activation(half[:], raw[:], AF.Copy, scale=0.5)
                bias[nm] = half
            dinvp = constp.tile([128, nwin], f32)
            nc.sync.dma_start(dinvp[:], dv_d[:])
            sqdeg = constp.tile([1, npc], bf16)
            nc.sync.dma_start(sqdeg[:], sq_d[:])

            g_all = constp.tile([128, npc], bf16)

            # idx streams fully preloaded (tiny); dma_gather slices them
            idx_all = []
            for h in range(n_halves):
                t = constp.tile([128, int(meta["half_tot"][h]) // 16], i16,
                                tag="idx%d" % h)
                nc.sync.dma_start(t[:], idx_d[h][:])
                idx_all.append(t)

            # one-hot slabs streamed from DRAM (fp8, OHG blocks per load).
            # lo/hi stream blocks interleave per window, so cache one live
            # slab per half to avoid thrashing.
            oh_view = oh_d[:].rearrange("p (b n) -> p b n", n=128)
            oh_cache = {}

            def get_oh(bg, h):
                g = bg // OHG
                if oh_cache.get(h, (None, None))[0] != g:
                    g0 = g * OHG
                    gl = min(OHG, nblk_tot - g0)
                    oh = ohp.tile([128, OHG, 128], fp8, tag="oh")
                    nc.scalar.dma_start(oh[:, :gl, :], oh_view[:, g0:g0 + gl, :])
                    oh_cache[h] = (g, oh)
                return oh_cache[h][1]

            # per-half stream state: lazy chunk issuing in window order
            class Stream:
                pass

            streams = []
            blk_base = 0
            for h in range(n_halves):
                s = Stream()
                s.h = h
                s.base = x_d[0:split, :] if h == 0 else x_d[split:n_nodes, :]
                s.wstart = np.concatenate([[0], np.cumsum(nblk[h])])
                s.blk_base = blk_base          # global block id of stream pos 0
                s.chunk_bounds = []
                off = 0
                for L in chunk_sizes[h]:
                    s.chunk_bounds.append((off, L))
                    off += L
                s.blk2chunk = np.repeat(
                    np.arange(len(chunk_sizes[h])),
                    [L // 128 for L in chunk_sizes[h]])
                s.tiles = {}
                blk_base += int(nblk[h].sum())
                streams.append(s)

            ci_global = 0

            def ensure_chunk(s, ci):
                nonlocal ci_global
                if ci in s.tiles:
                    return s.tiles[ci]
                off, L = s.chunk_bounds[ci]
                xg = xgp.tile([128, chunk // 128, 128], bf16, tag="xg")
                nc.gpsimd.dma_gather(
                    out_ap=xg[:, : L // 128, :],
                    in_ap=s.base,
                    idxs_ap=idx_all[s.h][:, off // 16:(off + L) // 16],
                    num_idxs=L,
                    num_idxs_reg=L,
                    elem_size=D,
                    single_packet=False,
                    queue_num=ci_global % N_QUEUES,
                )
                ci_global += 1
                s.tiles.clear()
                s.tiles[ci] = xg
                return xg

            # --- fused pass: per window accumulate lo+hi edge blocks in
            # one PSUM group, flush to g_all; every WG windows run the
            # output stage (overlaps with later windows' aggregation).
            def emit_phase2(wlo, whi):
                nwg = whi - wlo + 1
                wls = [min(WIN, npc - w * WIN) for w in range(wlo, whi + 1)]
                pps = {}
                for nm_w, nm_b in (("w1", "b1"), ("w2", "b2")):
                    pp = ps2.tile([128, WG * 128], f32, tag="pp")
                    for j, w in enumerate(range(wlo, whi + 1)):
                        wl = wls[j]
                        sl = pp[:wl, j * 128:(j + 1) * 128]
                        nc.tensor.matmul(sl, g_all[:, w * WIN:w * WIN + wl],
                                         wts[nm_w][:], start=True, stop=False)
                        nc.tensor.matmul(sl, sqdeg[:, w * WIN:w * WIN + wl],
                                         bias[nm_b][:], start=False, stop=True)
                    o = finp.tile([128, WG, 128], f32, tag="o" + nm_w)
                    # relu with per-partition dinv[dst] scale, per window
                    # (partition rows map to different nodes per column chunk)
                    for j, w in enumerate(range(wlo, whi + 1)):
                        nc.scalar.activation(
                            o[:wls[j], j, :],
                            pp[:wls[j], j * 128:(j + 1) * 128], AF.Relu,
                            scale=dinvp[:wls[j], w:w + 1])
                    pps[nm_w] = o
                ot = finp.tile([128, WG, 128], f32, tag="ot")
                otf = ot[:].rearrange("p c n -> p (c n)")
                o1f = pps["w1"][:].rearrange("p c n -> p (c n)")
                o2f = pps["w2"][:].rearrange("p c n -> p (c n)")
                rows = min(wls)
                if rows == 128:
                    nc.vector.tensor_tensor(otf[:, :nwg * 128],
                                            o1f[:, :nwg * 128],
                                            o2f[:, :nwg * 128], op=OP.add)
                else:
                    for j in range(nwg):
                        cs = slice(j * 128, j * 128 + 128)
                        nc.vector.tensor_tensor(otf[:wls[j], cs],
                                                o1f[:wls[j], cs],
                                                o2f[:wls[j], cs], op=OP.add)
                for j, w in enumerate(range(wlo, whi + 1)):
                    nc.sync.dma_start(out_d[w * WIN:w * WIN + wls[j], :],
                                      ot[:wls[j], j, :])

            for w in range(nwin):
                wlen = min(WIN, npc - w * WIN)
                pw = ps1.tile([128, 128], f32, tag="pw")
                # total blocks this window across halves
                runs = []
                for s in streams:
                    b0, b1 = int(s.wstart[w]), int(s.wstart[w + 1])
                    runs.append((s, b0, b1))
                n_tot = sum(b1 - b0 for _, b0, b1 in runs)
                k = 0
                for s, b0, b1 in runs:
                    for b in range(b0, b1):
                        ci = int(s.blk2chunk[b])
                        xg = ensure_chunk(s, ci)
                        bl = (b * 128 - s.chunk_bounds[ci][0]) // 128
                        bg = s.blk_base + b
                        oh = get_oh(bg, s.h)
                        nc.tensor.matmul(
                            pw[:, :wlen],
                            xg[:, bl, :],
                            oh[:, bg % OHG, :wlen],
                            start=(k == 0),
                            stop=(k == n_tot - 1),
                        )
                        k += 1
                nc.scalar.activation(g_all[:, w * WIN:w * WIN + wlen],
                                     pw[:, :wlen], AF.Copy)
                if w % WG == WG - 1 or w == nwin - 1:
                    emit_phase2(w - (w % WG), w)

    nc.compile()
    return nc


def make_core_inputs(meta, per_core_inputs, x, W1, b1, W2, b2):
    """Full in_maps for run_bass_kernel_spmd (adds shared tensors)."""
    import ml_dtypes
    xp = (np.asarray(x, np.float32)
          * meta["dinv_node"][:, None]).astype(ml_dtypes.bfloat16)
    shared = {
        "x": np.ascontiguousarray(xp),
        "W1": np.ascontiguousarray(np.asarray(W1, np.float32)),
        "W2": np.ascontiguousarray(np.asarray(W2, np.float32)),
        "b1": np.asarray(b1, np.float32).reshape(1, D),
        "b2": np.asarray(b2, np.float32).reshape(1, D),
    }
    return [dict(shared, **ci) for ci in per_core_inputs]


# ------------------------------------------------------------------- kernel

def kernel(x, edge_index, W1, b1, W2, b2, _trace=False):
    from concourse.bass_utils import run_bass_kernel_spmd

    x = np.asarray(x)
    n_nodes = x.shape[0]
    meta, pci = host_prep(edge_index, n_nodes, N_CORES)
    nc = build_program(meta)
    in_maps = make_core_inputs(meta, pci, x, W1, b1, W2, b2)
    res = run_bass_kernel_spmd(nc, in_maps, list(range(N_CORES)),
                               trace=_trace)
    out = np.concatenate([res.results[c]["out"] for c in range(N_CORES)],
                         axis=0)
    if _trace:
        return out, res
    return out


# revision 26
# speedup vs baseline: 1.6344x; 1.6344x over previous
"""Two-layer GCN (MultiOrderGraphLayer) Bass kernel for 8 Trainium2 cores.

Math: out = 0.5*(relu(A_hat@x@W1+b1) + relu(A_hat@x@W2+b2)) with
A_hat = D^-1/2 (A+I) D^-1/2.  Both layers share A_hat, so g = A_hat @ x is
computed once; the two small 128x128 matmuls run afterwards.

Normalization is split: norm_e = dinv[src]*dinv[dst] factors, so
  - host prescales x' = dinv (.) x  (bf16, one rounding),
  - the per-edge one-hot is a PURE 0/1 pattern (exact in fp8e4m3,
    precomputed on host, fully preloaded to SBUF -- no DVE work, no
    mid-loop DMA),
  - dinv[dst] is applied in phase 2 as a per-partition ACT scale on the
    relu (nodes are partitions there); the bias matmul uses a
    sqrt(deg) row instead of ones so relu(dinv*(gW + sqrt(deg)*b)) =
    relu(dinv*gW + b) exactly.

Edge layout (per core): edges partitioned by destination core, split into
lo/hi source streams (int16 gather index limit), each stream sorted by
128-node destination window but blocked WITHOUT per-window padding -- only
a single tail pad per stream (1.01x instead of 1.21x slots).  Blocks may
span window boundaries; the shared program runs each block against every
window it touches on ANY core (union ranges), and each (window, block)
pair gets its own one-hot column pattern, zero outside the window, so
cores with different boundaries stay correct.

Per block: dma_gather 128 rows of x' (256B bf16) and matmul
t_T += xg^T @ S into PSUM over the window's blocks; finish:
out[n, fo] = relu(dinv[n]*(g_T^T @ (0.5*W) + sqrt(deg)*0.5*b)) summed
over the two layers, written node-major straight from PSUM matmuls.
"""

import math
import numpy as np

N_NODES = 50000
D = 128
N_CORES = 8
SPLIT = 32768  # int16 gather index limit
WIN = 128      # output-window size in nodes (one-hot width / psum free dim)
CHUNK = 4096   # indices per dma_gather instruction (multiple of 128)
N_QUEUES = 4   # SWDGE queues; rotating queue_num 4x's gather bandwidth
OH_LOADS = 4   # preamble DMAs that fill the one-hot SBUF tile
WGRP = 8       # windows per padding group (streams re-anchor at group ends)


# ---------------------------------------------------------------- host prep

def host_prep(edge_index, n_nodes, n_cores, split=SPLIT, chunk=CHUNK):
    """Preprocessing: edge partitioning, window-sorted lo/hi streams with
    tail-only padding, shared per-window block ranges, 0/1 one-hot pattern
    per (window, block) pair, per-node dinv."""
    import ml_dtypes

    src = np.asarray(edge_index[0], dtype=np.int64)
    dst = np.asarray(edge_index[1], dtype=np.int64)
    deg = np.bincount(dst, minlength=n_nodes).astype(np.int64) + 1
    dinv_node = (1.0 / np.sqrt(deg)).astype(np.float32)
    sqdeg_node = np.sqrt(deg).astype(np.float32)

    # self-loops are NOT gathered: each window adds its own contiguous
    # x' slice via one identity matmul instead (phase 2's dinv[dst]
    # scale turns dinv[d]*x[d] into the dinv^2 self-loop term).
    s_all, d_all = src, dst

    npc = n_nodes // n_cores
    assert npc * n_cores == n_nodes
    nwin = math.ceil(npc / WIN)
    n_halves = 2 if n_nodes > split else 1

    core_streams = []  # [c][h] -> (srcrel, dloc, win)
    offs_all = np.zeros((n_cores, n_halves, nwin + 1), np.int64)
    for c in range(n_cores):
        n0 = c * npc
        m = (d_all >= n0) & (d_all < n0 + npc)
        s, d = s_all[m], d_all[m]
        w = (d - n0) // WIN
        half = (s >= split).astype(np.int64) if n_halves == 2 else np.zeros_like(s)
        per_h = []
        for h in range(n_halves):
            sel = half == h
            sh, dh, wh = s[sel], d[sel], w[sel]
            order = np.argsort(wh, kind="stable")
            sh, dh, wh = sh[order], dh[order], wh[order]
            cnt = np.bincount(wh, minlength=nwin)
            offs_all[c, h, 1:] = np.cumsum(cnt)
            per_h.append((sh - h * split, dh - n0 - wh * WIN, wh))
        core_streams.append(per_h)

    # group-anchored padding: streams padded to a shared length at the end
    # of every WGRP-window group, so per-core prefix drift cannot
    # accumulate; per-window block ranges are unions over cores within
    # the group.
    n_grp = -(-nwin // WGRP)
    glen = np.zeros((n_halves, n_grp), np.int64)
    gbase = np.zeros((n_halves, n_grp + 1), np.int64)
    for h in range(n_halves):
        for g in range(n_grp):
            gs_w, ge_w = g * WGRP, min((g + 1) * WGRP, nwin)
            L = (offs_all[:, h, ge_w] - offs_all[:, h, gs_w]).max()
            glen[h, g] = -(-L // 128) * 128
        gbase[h, 1:] = np.cumsum(glen[h])
    shared_len = [int(gbase[h, -1]) for h in range(n_halves)]
    # anchored per-core offsets -> shared block ranges per (half, window)
    wstart = np.zeros((n_cores, n_halves, nwin), np.int64)
    wend = np.zeros((n_cores, n_halves, nwin), np.int64)
    for h in range(n_halves):
        for g in range(n_grp):
            gs_w, ge_w = g * WGRP, min((g + 1) * WGRP, nwin)
            for v in range(gs_w, ge_w):
                wstart[:, h, v] = gbase[h, g] + (offs_all[:, h, v]
                                                 - offs_all[:, h, gs_w])
                wend[:, h, v] = gbase[h, g] + (offs_all[:, h, v + 1]
                                               - offs_all[:, h, gs_w])
    bs = np.zeros((n_halves, nwin), np.int64)
    be = np.zeros((n_halves, nwin), np.int64)
    for h in range(n_halves):
        bs[h] = wstart[:, h, :].min(axis=0) // 128
        be[h] = np.maximum(-(-wend[:, h, :].max(axis=0) // 128), bs[h])
    seq_tot = int((be - bs).sum())

    # chunk split per half (shared across cores); warmup chunks so the
    # first PSUM windows start quickly
    chunk_sizes = []
    for h in range(n_halves):
        rem, sizes = shared_len[h], []
        for warm in (1024, 1024, 2048, 2048):
            L = min(warm, rem)
            if L > 0:
                sizes.append(L)
                rem -= L
        while rem > 0:
            L = min(chunk, rem)
            sizes.append(L)
            rem -= L
        chunk_sizes.append(sizes)

    per_core_inputs = []
    lanes = np.arange(128)
    for c in range(n_cores):
        slot_src, slot_dl, slot_win = [], [], []
        for h in range(n_halves):
            sh, dlh, wh = core_streams[c][h]
            ss = np.zeros(shared_len[h], np.int64)
            sd = np.full(shared_len[h], -1, np.int64)
            sw = np.full(shared_len[h], -1, np.int64)
            for g in range(n_grp):
                gs_w, ge_w = g * WGRP, min((g + 1) * WGRP, nwin)
                a = offs_all[c, h, gs_w]
                b = offs_all[c, h, ge_w]
                gb = int(gbase[h, g])
                ss[gb:gb + (b - a)] = sh[a:b]
                sd[gb:gb + (b - a)] = dlh[a:b]
                sw[gb:gb + (b - a)] = wh[a:b]
            slot_src.append(ss)
            slot_dl.append(sd)
            slot_win.append(sw)

        # one-hot pattern per emission seq: for v: for h: for b in [bs, be)
        ohp = np.zeros((128, seq_tot, 128), dtype=ml_dtypes.float8_e4m3)
        seq = 0
        for v in range(nwin):
            for h in range(n_halves):
                for b in range(int(bs[h][v]), int(be[h][v])):
                    sl = slice(b * 128, (b + 1) * 128)
                    msk = slot_win[h][sl] == v
                    ohp[lanes[msk], seq, slot_dl[h][sl][msk]] = 1.0
                    seq += 1
        assert seq == seq_tot

        dv = np.ones((128, nwin), np.float32)
        for wi in range(nwin):
            wl = min(WIN, npc - wi * WIN)
            dv[:wl, wi] = dinv_node[c * npc + wi * WIN:c * npc + wi * WIN + wl]
        core_in = {
            "ohpat": np.ascontiguousarray(ohp.reshape(128, seq_tot * 128)),
            "dinvp": dv,
            "sqdeg": np.ascontiguousarray(
                sqdeg_node[c * npc:(c + 1) * npc].astype(ml_dtypes.bfloat16)
            ).reshape(1, npc),
        }
        # gather indices: wrapped [16, L/16] per chunk, replicated to 128 rows
        for h in range(n_halves):
            stream = slot_src[h].astype(np.int16)
            cols, off = [], 0
            for L in chunk_sizes[h]:
                a = stream[off:off + L].reshape(-1, 16).T
                cols.append(a)
                off += L
            wrapped = np.concatenate(cols, axis=1)
            core_in["idx_h%d" % h] = np.ascontiguousarray(np.tile(wrapped, (8, 1)))
        per_core_inputs.append(core_in)

    meta = dict(n_nodes=n_nodes, n_cores=n_cores, npc=npc, nwin=nwin,
                n_halves=n_halves, split=split, bs=bs, be=be,
                shared_len=shared_len, seq_tot=seq_tot, chunk=chunk,
                chunk_sizes=chunk_sizes, dinv_node=dinv_node)
    return meta, per_core_inputs


# ------------------------------------------------------------- bass program

def build_program(meta):
    import concourse.bacc as bacc
    import concourse.mybir as mybir
    import concourse.tile as tile
    from concourse import library_config

    f32 = mybir.dt.float32
    bf16 = mybir.dt.bfloat16
    fp8 = mybir.dt.float8e4
    i16 = mybir.dt.int16
    AF = mybir.ActivationFunctionType
    OP = mybir.AluOpType

    n_nodes = meta["n_nodes"]
    npc, nwin = meta["npc"], meta["nwin"]
    n_halves, split = meta["n_halves"], meta["split"]
    bs, be = meta["bs"], meta["be"]
    seq_tot = meta["seq_tot"]
    chunk = meta["chunk"]
    chunk_sizes = meta["chunk_sizes"]

    nc = bacc.Bacc("TRN2", num_swdge_queues=N_QUEUES)

    x_d = nc.declare_dram_parameter("x", [n_nodes, D], bf16, isOutput=False)
    oh_d = nc.declare_dram_parameter("ohpat", [128, seq_tot * 128], fp8,
                                     isOutput=False)
    dv_d = nc.declare_dram_parameter("dinvp", [128, nwin], f32, isOutput=False)
    sq_d = nc.declare_dram_parameter("sqdeg", [1, npc], bf16, isOutput=False)
    idx_d = [nc.declare_dram_parameter("idx_h%d" % h,
                                       [128, meta["shared_len"][h] // 16],
                                       i16, isOutput=False)
             for h in range(n_halves)]
    w1_d = nc.declare_dram_parameter("W1", [D, D], f32, isOutput=False)
    w2_d = nc.declare_dram_parameter("W2", [D, D], f32, isOutput=False)
    b1_d = nc.declare_dram_parameter("b1", [1, D], f32, isOutput=False)
    b2_d = nc.declare_dram_parameter("b2", [1, D], f32, isOutput=False)
    xo_d = nc.declare_dram_parameter("xown", [npc, D], bf16, isOutput=False)
    id_d = nc.declare_dram_parameter("ident", [128, 128], fp8, isOutput=False)
    out_d = nc.declare_dram_parameter("out", [npc, D], f32, isOutput=True)

    WG = 4  # windows per phase-2 batch (one 512-wide psum bank)

    with tile.TileContext(nc) as tc:
        with (
            tc.tile_pool(name="const", bufs=1) as constp,
            tc.tile_pool(name="xg", bufs=5) as xgp,
            tc.tile_pool(name="ps1", bufs=3, space="PSUM") as ps1,
            tc.tile_pool(name="ps2", bufs=2, space="PSUM") as ps2,
            tc.tile_pool(name="fin", bufs=2) as finp,
        ):
            # Q7 library holding DMAGatherAnt; must precede all gathers
            nc.gpsimd.load_library(library_config.mlp)

            # --- constants / metadata
            wts = {}
            for nm, src_d in (("w1", w1_d), ("w2", w2_d)):
                raw = constp.tile([128, 128], f32, tag=nm + "raw")
                nc.sync.dma_start(raw[:], src_d[:])
                half = constp.tile([128, 128], bf16, tag=nm + "half")
                nc.scalar.activation(half[:], raw[:], AF.Copy, scale=0.5)
                wts[nm] = half
            bias = {}
            for nm, src_d in (("b1", b1_d), ("b2", b2_d)):
                raw = constp.tile([1, 128], f32, tag=nm + "raw")
                nc.sync.dma_start(raw[:], src_d[:])
                half = constp.tile([1, 128], bf16, tag=nm + "half")
                nc.scalar.activation(half[:], raw[:], AF.Copy, scale=0.5)
                bias[nm] = half
            dinvp = constp.tile([128, nwin], f32)
            nc.sync.dma_start(dinvp[:], dv_d[:])
            sqdeg = constp.tile([1, npc], bf16)
            nc.sync.dma_start(sqdeg[:], sq_d[:])
            ident = constp.tile([128, 128], fp8)
            nc.sync.dma_start(ident[:], id_d[:])

            # own-slice x'' window-major for the self-loop identity matmul
            xow = constp.tile([128, nwin, 128], bf16)
            nfull = npc // 128
            nc.sync.dma_start(
                xow[:, :nfull, :],
                xo_d[0:nfull * 128, :].rearrange("(w p) f -> p w f", p=128))
            if npc % 128:
                nc.sync.dma_start(xow[: npc % 128, nfull, :],
                                  xo_d[nfull * 128:npc, :])

            g_all = constp.tile([128, npc], bf16)

            # idx streams fully preloaded (tiny); dma_gather slices them
            idx_all = []
            for h in range(n_halves):
                t = constp.tile([128, meta["shared_len"][h] // 16], i16,
                                tag="idx%d" % h)
                nc.sync.dma_start(t[:], idx_d[h][:])
                idx_all.append(t)

            # full one-hot stream preloaded to SBUF in a few big
            # line-rate DMAs; matmuls slice it directly
            oh_all = constp.tile([128, seq_tot, 128], fp8)
            oh_view = oh_d[:].rearrange("p (b n) -> p b n", n=128)
            step = -(-seq_tot // OH_LOADS)
            for a in range(0, seq_tot, step):
                z = min(seq_tot, a + step)
                nc.sync.dma_start(oh_all[:, a:z, :], oh_view[:, a:z, :])

            # per-half stream state: lazy chunk issuing in window order
            class Stream:
                pass

            streams = []
            for h in range(n_halves):
                s = Stream()
                s.h = h
                s.base = x_d[0:split, :] if h == 0 else x_d[split:n_nodes, :]
                s.chunk_bounds = []
                off = 0
                for L in chunk_sizes[h]:
                    s.chunk_bounds.append((off, L))
                    off += L
                s.blk2chunk = np.repeat(
                    np.arange(len(chunk_sizes[h])),
                    [L // 128 for L in chunk_sizes[h]])
                s.tiles = {}
                streams.append(s)

            ci_global = 0

            def ensure_chunk(s, ci):
                nonlocal ci_global
                if ci in s.tiles:
                    return s.tiles[ci]
                off, L = s.chunk_bounds[ci]
                xg = xgp.tile([128, chunk // 128, 128], bf16, tag="xg")
                nc.gpsimd.dma_gather(
                    out_ap=xg[:, : L // 128, :],
                    in_ap=s.base,
                    idxs_ap=idx_all[s.h][:, off // 16:(off + L) // 16],
                    num_idxs=L,
                    num_idxs_reg=L,
                    elem_size=D,
                    single_packet=False,
                    queue_num=ci_global % N_QUEUES,
                )
                ci_global += 1
                s.tiles.clear()
                s.tiles[ci] = xg
                return xg

            # --- output stage: every WG windows, two weight matmuls +
            # per-window dinv-scaled relu + combine; overlaps aggregation
            def emit_phase2(wlo, whi):
                nwg = whi - wlo + 1
                wls = [min(WIN, npc - w * WIN) for w in range(wlo, whi + 1)]
                pps = {}
                for nm_w, nm_b in (("w1", "b1"), ("w2", "b2")):
                    pp = ps2.tile([128, WG * 128], f32, tag="pp")
                    for j, w in enumerate(range(wlo, whi + 1)):
                        wl = wls[j]
                        sl = pp[:wl, j * 128:(j + 1) * 128]
                        nc.tensor.matmul(sl, g_all[:, w * WIN:w * WIN + wl],
                                         wts[nm_w][:], start=True, stop=False)
                        nc.tensor.matmul(sl, sqdeg[:, w * WIN:w * WIN + wl],
                                         bias[nm_b][:], start=False, stop=True)
                    o = finp.tile([128, WG, 128], f32, tag="o" + nm_w)
                    for j, w in enumerate(range(wlo, whi + 1)):
                        nc.scalar.activation(
                            o[:wls[j], j, :],
                            pp[:wls[j], j * 128:(j + 1) * 128], AF.Relu,
                            scale=dinvp[:wls[j], w:w + 1])
                    pps[nm_w] = o
                ot = finp.tile([128, WG, 128], f32, tag="ot")
                otf = ot[:].rearrange("p c n -> p (c n)")
                o1f = pps["w1"][:].rearrange("p c n -> p (c n)")
                o2f = pps["w2"][:].rearrange("p c n -> p (c n)")
                rows = min(wls)
                if rows == 128:
                    nc.vector.tensor_tensor(otf[:, :nwg * 128],
                                            o1f[:, :nwg * 128],
                                            o2f[:, :nwg * 128], op=OP.add)
                else:
                    for j in range(nwg):
                        cs = slice(j * 128, j * 128 + 128)
                        nc.vector.tensor_tensor(otf[:wls[j], cs],
                                                o1f[:wls[j], cs],
                                                o2f[:wls[j], cs], op=OP.add)
                for j, w in enumerate(range(wlo, whi + 1)):
                    nc.sync.dma_start(out_d[w * WIN:w * WIN + wls[j], :],
                                      ot[:wls[j], j, :])

            seq = 0
            for w in range(nwin):
                wlen = min(WIN, npc - w * WIN)
                pw = ps1.tile([128, 128], f32, tag="pw")
                n_tot = 1 + sum(int(be[s.h][w] - bs[s.h][w]) for s in streams)
                # self-loop: pw += xown_w^T @ I  (exact fp8 identity)
                nc.tensor.matmul(pw[:, :wlen], xow[:wlen, w, :],
                                 ident[:wlen, :wlen],
                                 start=True, stop=(n_tot == 1))
                k = 1
                for s in streams:
                    for b in range(int(bs[s.h][w]), int(be[s.h][w])):
                        ci = int(s.blk2chunk[b])
                        xg = ensure_chunk(s, ci)
                        bl = (b * 128 - s.chunk_bounds[ci][0]) // 128
                        nc.tensor.matmul(
                            pw[:, :wlen],
                            xg[:, bl, :],
                            oh_all[:, seq, :wlen],
                            start=False,
                            stop=(k == n_tot - 1),
                        )
                        seq += 1
                        k += 1
                nc.scalar.activation(g_all[:, w * WIN:w * WIN + wlen],
                                     pw[:, :wlen], AF.Copy)
                if w % WG == WG - 1 or w == nwin - 1:
                    emit_phase2(w - (w % WG), w)
            assert seq == seq_tot

    nc.compile()
    return nc


def make_core_inputs(meta, per_core_inputs, x, W1, b1, W2, b2):
    """Full in_maps for run_bass_kernel_spmd (adds shared tensors)."""
    import ml_dtypes
    xf = np.asarray(x, np.float32)
    dinv = meta["dinv_node"]
    xp = (xf * dinv[:, None]).astype(ml_dtypes.bfloat16)
    npc = meta["npc"]
    shared = {
        "x": np.ascontiguousarray(xp),
        "W1": np.ascontiguousarray(np.asarray(W1, np.float32)),
        "W2": np.ascontiguousarray(np.asarray(W2, np.float32)),
        "b1": np.asarray(b1, np.float32).reshape(1, D),
        "b2": np.asarray(b2, np.float32).reshape(1, D),
        "ident": np.ascontiguousarray(
            np.eye(128, dtype=ml_dtypes.float8_e4m3)),
    }
    maps = []
    for c, ci in enumerate(per_core_inputs):
        ci = dict(ci)
        ci["xown"] = np.ascontiguousarray(xp[c * npc:(c + 1) * npc])
        maps.append(dict(shared, **ci))
    return maps


# ------------------------------------------------------------------- kernel

def kernel(x, edge_index, W1, b1, W2, b2, _trace=False):
    from concourse.bass_utils import run_bass_kernel_spmd

    x = np.asarray(x)
    n_nodes = x.shape[0]
    meta, pci = host_prep(edge_index, n_nodes, N_CORES)
    nc = build_program(meta)
    in_maps = make_core_inputs(meta, pci, x, W1, b1, W2, b2)
    res = run_bass_kernel_spmd(nc, in_maps, list(range(N_CORES)),
                               trace=_trace)
    out = np.concatenate([res.results[c]["out"] for c in range(N_CORES)],
                         axis=0)
    if _trace:
        return out, res
    return out


# revision 31
# speedup vs baseline: 1.7314x; 1.0594x over previous
"""Two-layer GCN (MultiOrderGraphLayer) Bass kernel for 8 Trainium2 cores.

Math: out = 0.5*(relu(A_hat@x@W1+b1) + relu(A_hat@x@W2+b2)) with
A_hat = D^-1/2 (A+I) D^-1/2.  Both layers share A_hat, so g = A_hat @ x is
computed once; the two small 128x128 matmuls run afterwards.

Normalization is split: norm_e = dinv[src]*dinv[dst] factors, so
  - host prescales x' = dinv (.) x  (bf16, one rounding),
  - the per-edge one-hot is a PURE 0/1 pattern (exact in fp8e4m3,
    precomputed on host, fully preloaded to SBUF -- no DVE work, no
    mid-loop DMA),
  - dinv[dst] is applied in phase 2 as a per-partition ACT scale on the
    relu (nodes are partitions there); the bias matmul uses a
    sqrt(deg) row instead of ones so relu(dinv*(gW + sqrt(deg)*b)) =
    relu(dinv*gW + b) exactly.

Edge layout (per core): edges partitioned by destination core, split into
lo/hi source streams (int16 gather index limit), each stream sorted by
128-node destination window but blocked WITHOUT per-window padding -- only
a single tail pad per stream (1.01x instead of 1.21x slots).  Blocks may
span window boundaries; the shared program runs each block against every
window it touches on ANY core (union ranges), and each (window, block)
pair gets its own one-hot column pattern, zero outside the window, so
cores with different boundaries stay correct.

Per block: dma_gather 128 rows of x' (256B bf16) and matmul
t_T += xg^T @ S into PSUM over the window's blocks; finish:
out[n, fo] = relu(dinv[n]*(g_T^T @ (0.5*W) + sqrt(deg)*0.5*b)) summed
over the two layers, written node-major straight from PSUM matmuls.
"""

import math
import numpy as np

N_NODES = 50000
D = 128
N_CORES = 8
SPLIT = 32768  # int16 gather index limit
WIN = 128      # output-window size in nodes (one-hot width / psum free dim)
CHUNK = 1024   # indices per dma_gather instruction (multiple of 128);
               # small chunks keep all 4 SWDGE rings fed so their drains
               # overlap (one big chunk overflows its ring and serializes)
N_QUEUES = 4   # SWDGE queues; rotating queue_num 4x's gather bandwidth
OH_LOADS = 4   # preamble DMAs that fill the one-hot SBUF tile
WGRP = 8       # windows per padding group (streams re-anchor at group ends)


# ---------------------------------------------------------------- host prep

def host_prep(edge_index, n_nodes, n_cores, split=SPLIT, chunk=CHUNK):
    """Preprocessing: edge partitioning, window-sorted lo/hi streams with
    tail-only padding, shared per-window block ranges, 0/1 one-hot pattern
    per (window, block) pair, per-node dinv."""
    import ml_dtypes

    src = np.asarray(edge_index[0], dtype=np.int64)
    dst = np.asarray(edge_index[1], dtype=np.int64)
    deg = np.bincount(dst, minlength=n_nodes).astype(np.int64) + 1
    dinv_node = (1.0 / np.sqrt(deg)).astype(np.float32)
    sqdeg_node = np.sqrt(deg).astype(np.float32)

    # self-loops are NOT gathered: each window adds its own contiguous
    # x' slice via one identity matmul instead (phase 2's dinv[dst]
    # scale turns dinv[d]*x[d] into the dinv^2 self-loop term).
    s_all, d_all = src, dst

    npc = n_nodes // n_cores
    assert npc * n_cores == n_nodes
    nwin = math.ceil(npc / WIN)
    n_halves = 2 if n_nodes > split else 1

    core_streams = []  # [c][h] -> (srcrel, dloc, win)
    offs_all = np.zeros((n_cores, n_halves, nwin + 1), np.int64)
    for c in range(n_cores):
        n0 = c * npc
        m = (d_all >= n0) & (d_all < n0 + npc)
        s, d = s_all[m], d_all[m]
        w = (d - n0) // WIN
        half = (s >= split).astype(np.int64) if n_halves == 2 else np.zeros_like(s)
        per_h = []
        for h in range(n_halves):
            sel = half == h
            sh, dh, wh = s[sel], d[sel], w[sel]
            order = np.argsort(wh, kind="stable")
            sh, dh, wh = sh[order], dh[order], wh[order]
            cnt = np.bincount(wh, minlength=nwin)
            offs_all[c, h, 1:] = np.cumsum(cnt)
            per_h.append((sh - h * split, dh - n0 - wh * WIN, wh))
        core_streams.append(per_h)

    # group-anchored padding: streams padded to a shared length at the end
    # of every WGRP-window group, so per-core prefix drift cannot
    # accumulate; per-window block ranges are unions over cores within
    # the group.
    n_grp = -(-nwin // WGRP)
    glen = np.zeros((n_halves, n_grp), np.int64)
    gbase = np.zeros((n_halves, n_grp + 1), np.int64)
    for h in range(n_halves):
        for g in range(n_grp):
            gs_w, ge_w = g * WGRP, min((g + 1) * WGRP, nwin)
            L = (offs_all[:, h, ge_w] - offs_all[:, h, gs_w]).max()
            glen[h, g] = -(-L // 128) * 128
        gbase[h, 1:] = np.cumsum(glen[h])
    shared_len = [int(gbase[h, -1]) for h in range(n_halves)]
    # anchored per-core offsets -> shared block ranges per (half, window)
    wstart = np.zeros((n_cores, n_halves, nwin), np.int64)
    wend = np.zeros((n_cores, n_halves, nwin), np.int64)
    for h in range(n_halves):
        for g in range(n_grp):
            gs_w, ge_w = g * WGRP, min((g + 1) * WGRP, nwin)
            for v in range(gs_w, ge_w):
                wstart[:, h, v] = gbase[h, g] + (offs_all[:, h, v]
                                                 - offs_all[:, h, gs_w])
                wend[:, h, v] = gbase[h, g] + (offs_all[:, h, v + 1]
                                               - offs_all[:, h, gs_w])
    bs = np.zeros((n_halves, nwin), np.int64)
    be = np.zeros((n_halves, nwin), np.int64)
    for h in range(n_halves):
        bs[h] = wstart[:, h, :].min(axis=0) // 128
        be[h] = np.maximum(-(-wend[:, h, :].max(axis=0) // 128), bs[h])
    seq_tot = int((be - bs).sum())

    # chunk split per half (shared across cores)
    chunk_sizes = []
    for h in range(n_halves):
        rem, sizes = shared_len[h], []
        while rem > 0:
            L = min(chunk, rem)
            sizes.append(L)
            rem -= L
        chunk_sizes.append(sizes)

    per_core_inputs = []
    lanes = np.arange(128)
    for c in range(n_cores):
        slot_src, slot_dl, slot_win = [], [], []
        for h in range(n_halves):
            sh, dlh, wh = core_streams[c][h]
            ss = np.zeros(shared_len[h], np.int64)
            sd = np.full(shared_len[h], -1, np.int64)
            sw = np.full(shared_len[h], -1, np.int64)
            for g in range(n_grp):
                gs_w, ge_w = g * WGRP, min((g + 1) * WGRP, nwin)
                a = offs_all[c, h, gs_w]
                b = offs_all[c, h, ge_w]
                gb = int(gbase[h, g])
                ss[gb:gb + (b - a)] = sh[a:b]
                sd[gb:gb + (b - a)] = dlh[a:b]
                sw[gb:gb + (b - a)] = wh[a:b]
            slot_src.append(ss)
            slot_dl.append(sd)
            slot_win.append(sw)

        # one-hot pattern per emission seq: for v: for h: for b in [bs, be)
        ohp = np.zeros((128, seq_tot, 128), dtype=ml_dtypes.float8_e4m3)
        seq = 0
        for v in range(nwin):
            for h in range(n_halves):
                for b in range(int(bs[h][v]), int(be[h][v])):
                    sl = slice(b * 128, (b + 1) * 128)
                    msk = slot_win[h][sl] == v
                    ohp[lanes[msk], seq, slot_dl[h][sl][msk]] = 1.0
                    seq += 1
        assert seq == seq_tot

        dv = np.ones((128, nwin), np.float32)
        for wi in range(nwin):
            wl = min(WIN, npc - wi * WIN)
            dv[:wl, wi] = dinv_node[c * npc + wi * WIN:c * npc + wi * WIN + wl]
        core_in = {
            "ohpat": np.ascontiguousarray(ohp.reshape(128, seq_tot * 128)),
            "dinvp": dv,
            "sqdeg": np.ascontiguousarray(
                sqdeg_node[c * npc:(c + 1) * npc].astype(ml_dtypes.bfloat16)
            ).reshape(1, npc),
        }
        # gather indices: wrapped [16, L/16] per chunk, replicated to 128 rows
        for h in range(n_halves):
            stream = slot_src[h].astype(np.int16)
            cols, off = [], 0
            for L in chunk_sizes[h]:
                a = stream[off:off + L].reshape(-1, 16).T
                cols.append(a)
                off += L
            wrapped = np.concatenate(cols, axis=1)
            core_in["idx_h%d" % h] = np.ascontiguousarray(np.tile(wrapped, (8, 1)))
        per_core_inputs.append(core_in)

    meta = dict(n_nodes=n_nodes, n_cores=n_cores, npc=npc, nwin=nwin,
                n_halves=n_halves, split=split, bs=bs, be=be,
                shared_len=shared_len, seq_tot=seq_tot, chunk=chunk,
                chunk_sizes=chunk_sizes, dinv_node=dinv_node)
    return meta, per_core_inputs


# ------------------------------------------------------------- bass program

def build_program(meta):
    import concourse.bacc as bacc
    import concourse.mybir as mybir
    import concourse.tile as tile
    from concourse import library_config

    f32 = mybir.dt.float32
    bf16 = mybir.dt.bfloat16
    fp8 = mybir.dt.float8e4
    i16 = mybir.dt.int16
    AF = mybir.ActivationFunctionType
    OP = mybir.AluOpType

    n_nodes = meta["n_nodes"]
    npc, nwin = meta["npc"], meta["nwin"]
    n_halves, split = meta["n_halves"], meta["split"]
    bs, be = meta["bs"], meta["be"]
    seq_tot = meta["seq_tot"]
    chunk = meta["chunk"]
    chunk_sizes = meta["chunk_sizes"]

    nc = bacc.Bacc("TRN2", num_swdge_queues=N_QUEUES)

    x_d = nc.declare_dram_parameter("x", [n_nodes, D], bf16, isOutput=False)
    oh_d = nc.declare_dram_parameter("ohpat", [128, seq_tot * 128], fp8,
                                     isOutput=False)
    dv_d = nc.declare_dram_parameter("dinvp", [128, nwin], f32, isOutput=False)
    sq_d = nc.declare_dram_parameter("sqdeg", [1, npc], bf16, isOutput=False)
    idx_d = [nc.declare_dram_parameter("idx_h%d" % h,
                                       [128, meta["shared_len"][h] // 16],
                                       i16, isOutput=False)
             for h in range(n_halves)]
    w1_d = nc.declare_dram_parameter("W1", [D, D], f32, isOutput=False)
    w2_d = nc.declare_dram_parameter("W2", [D, D], f32, isOutput=False)
    b1_d = nc.declare_dram_parameter("b1", [1, D], f32, isOutput=False)
    b2_d = nc.declare_dram_parameter("b2", [1, D], f32, isOutput=False)
    xo_d = nc.declare_dram_parameter("xown", [npc, D], bf16, isOutput=False)
    id_d = nc.declare_dram_parameter("ident", [128, 128], fp8, isOutput=False)
    out_d = nc.declare_dram_parameter("out", [npc, D], f32, isOutput=True)

    WG = 4  # windows per phase-2 batch (one 512-wide psum bank)

    with tile.TileContext(nc) as tc:
        with (
            tc.tile_pool(name="const", bufs=1) as constp,
            tc.tile_pool(name="xg", bufs=10) as xgp,
            tc.tile_pool(name="ps1", bufs=3, space="PSUM") as ps1,
            tc.tile_pool(name="ps2", bufs=2, space="PSUM") as ps2,
            tc.tile_pool(name="fin", bufs=2) as finp,
        ):
            # Q7 library holding DMAGatherAnt; must precede all gathers
            nc.gpsimd.load_library(library_config.mlp)

            # idx streams first on the sync ring so gathers start ASAP
            idx_all = []
            for h in range(n_halves):
                t = constp.tile([128, meta["shared_len"][h] // 16], i16,
                                tag="idx%d" % h)
                nc.sync.dma_start(t[:], idx_d[h][:])
                idx_all.append(t)

            # big one-hot / xown loads on the scalar ring, in parallel
            # with the sync ring's preamble
            oh_all = constp.tile([128, seq_tot, 128], fp8)
            oh_view = oh_d[:].rearrange("p (b n) -> p b n", n=128)
            step = -(-seq_tot // OH_LOADS)
            for a in range(0, seq_tot, step):
                z = min(seq_tot, a + step)
                nc.scalar.dma_start(oh_all[:, a:z, :], oh_view[:, a:z, :])
            xow = constp.tile([128, nwin, 128], bf16)
            nfull = npc // 128
            nc.scalar.dma_start(
                xow[:, :nfull, :],
                xo_d[0:nfull * 128, :].rearrange("(w p) f -> p w f", p=128))
            if npc % 128:
                nc.scalar.dma_start(xow[: npc % 128, nfull, :],
                                    xo_d[nfull * 128:npc, :])

            # --- constants / metadata
            wts = {}
            for nm, src_d in (("w1", w1_d), ("w2", w2_d)):
                raw = constp.tile([128, 128], f32, tag=nm + "raw")
                nc.sync.dma_start(raw[:], src_d[:])
                half = constp.tile([128, 128], bf16, tag=nm + "half")
                nc.scalar.activation(half[:], raw[:], AF.Copy, scale=0.5)
                wts[nm] = half
            bias = {}
            for nm, src_d in (("b1", b1_d), ("b2", b2_d)):
                raw = constp.tile([1, 128], f32, tag=nm + "raw")
                nc.sync.dma_start(raw[:], src_d[:])
                half = constp.tile([1, 128], bf16, tag=nm + "half")
                nc.scalar.activation(half[:], raw[:], AF.Copy, scale=0.5)
                bias[nm] = half
            dinvp = constp.tile([128, nwin], f32)
            nc.sync.dma_start(dinvp[:], dv_d[:])
            sqdeg = constp.tile([1, npc], bf16)
            nc.sync.dma_start(sqdeg[:], sq_d[:])
            ident = constp.tile([128, 128], fp8)
            nc.sync.dma_start(ident[:], id_d[:])

            g_all = constp.tile([128, npc], bf16)

            # per-half stream state: lazy chunk issuing in window order
            class Stream:
                pass

            streams = []
            for h in range(n_halves):
                s = Stream()
                s.h = h
                s.base = x_d[0:split, :] if h == 0 else x_d[split:n_nodes, :]
                s.chunk_bounds = []
                off = 0
                for L in chunk_sizes[h]:
                    s.chunk_bounds.append((off, L))
                    off += L
                s.blk2chunk = np.repeat(
                    np.arange(len(chunk_sizes[h])),
                    [L // 128 for L in chunk_sizes[h]])
                s.tiles = {}
                streams.append(s)

            ci_global = 0

            def ensure_chunk(s, ci):
                nonlocal ci_global
                if ci in s.tiles:
                    return s.tiles[ci]
                off, L = s.chunk_bounds[ci]
                xg = xgp.tile([128, chunk // 128, 128], bf16, tag="xg")
                nc.gpsimd.dma_gather(
                    out_ap=xg[:, : L // 128, :],
                    in_ap=s.base,
                    idxs_ap=idx_all[s.h][:, off // 16:(off + L) // 16],
                    num_idxs=L,
                    num_idxs_reg=L,
                    elem_size=D,
                    single_packet=False,
                    queue_num=ci_global % N_QUEUES,
                )
                ci_global += 1
                s.tiles.clear()
                s.tiles[ci] = xg
                return xg

            # --- output stage: every WG windows, two weight matmuls +
            # per-window dinv-scaled relu + combine; overlaps aggregation
            def emit_phase2(wlo, whi):
                nwg = whi - wlo + 1
                wls = [min(WIN, npc - w * WIN) for w in range(wlo, whi + 1)]
                pps = {}
                for nm_w, nm_b in (("w1", "b1"), ("w2", "b2")):
                    pp = ps2.tile([128, WG * 128], f32, tag="pp")
                    for j, w in enumerate(range(wlo, whi + 1)):
                        wl = wls[j]
                        sl = pp[:wl, j * 128:(j + 1) * 128]
                        nc.tensor.matmul(sl, g_all[:, w * WIN:w * WIN + wl],
                                         wts[nm_w][:], start=True, stop=False)
                        nc.tensor.matmul(sl, sqdeg[:, w * WIN:w * WIN + wl],
                                         bias[nm_b][:], start=False, stop=True)
                    o = finp.tile([128, WG, 128], f32, tag="o" + nm_w)
                    for j, w in enumerate(range(wlo, whi + 1)):
                        nc.scalar.activation(
                            o[:wls[j], j, :],
                            pp[:wls[j], j * 128:(j + 1) * 128], AF.Relu,
                            scale=dinvp[:wls[j], w:w + 1])
                    pps[nm_w] = o
                ot = finp.tile([128, WG, 128], f32, tag="ot")
                otf = ot[:].rearrange("p c n -> p (c n)")
                o1f = pps["w1"][:].rearrange("p c n -> p (c n)")
                o2f = pps["w2"][:].rearrange("p c n -> p (c n)")
                rows = min(wls)
                if rows == 128:
                    nc.vector.tensor_tensor(otf[:, :nwg * 128],
                                            o1f[:, :nwg * 128],
                                            o2f[:, :nwg * 128], op=OP.add)
                else:
                    for j in range(nwg):
                        cs = slice(j * 128, j * 128 + 128)
                        nc.vector.tensor_tensor(otf[:wls[j], cs],
                                                o1f[:wls[j], cs],
                                                o2f[:wls[j], cs], op=OP.add)
                for j, w in enumerate(range(wlo, whi + 1)):
                    nc.sync.dma_start(out_d[w * WIN:w * WIN + wls[j], :],
                                      ot[:wls[j], j, :])

            seq = 0
            for w in range(nwin):
                wlen = min(WIN, npc - w * WIN)
                pw = ps1.tile([128, 128], f32, tag="pw")
                n_tot = 1 + sum(int(be[s.h][w] - bs[s.h][w]) for s in streams)
                # self-loop: pw += xown_w^T @ I  (exact fp8 identity)
                nc.tensor.matmul(pw[:, :wlen], xow[:wlen, w, :],
                                 ident[:wlen, :wlen],
                                 start=True, stop=(n_tot == 1))
                k = 1
                for s in streams:
                    for b in range(int(bs[s.h][w]), int(be[s.h][w])):
                        ci = int(s.blk2chunk[b])
                        xg = ensure_chunk(s, ci)
                        bl = (b * 128 - s.chunk_bounds[ci][0]) // 128
                        nc.tensor.matmul(
                            pw[:, :wlen],
                            xg[:, bl, :],
                            oh_all[:, seq, :wlen],
                            start=False,
                            stop=(k == n_tot - 1),
                        )
                        seq += 1
                        k += 1
                nc.scalar.activation(g_all[:, w * WIN:w * WIN + wlen],
                                     pw[:, :wlen], AF.Copy)
                if w % WG == WG - 1 or w == nwin - 1:
                    emit_phase2(w - (w % WG), w)
            assert seq == seq_tot

    nc.compile()
    return nc


def make_core_inputs(meta, per_core_inputs, x, W1, b1, W2, b2):
    """Full in_maps for run_bass_kernel_spmd (adds shared tensors)."""
    import ml_dtypes
    xf = np.asarray(x, np.float32)
    dinv = meta["dinv_node"]
    xp = (xf * dinv[:, None]).astype(ml_dtypes.bfloat16)
    npc = meta["npc"]
    shared = {
        "x": np.ascontiguousarray(xp),
        "W1": np.ascontiguousarray(np.asarray(W1, np.float32)),
        "W2": np.ascontiguousarray(np.asarray(W2, np.float32)),
        "b1": np.asarray(b1, np.float32).reshape(1, D),
        "b2": np.asarray(b2, np.float32).reshape(1, D),
        "ident": np.ascontiguousarray(
            np.eye(128, dtype=ml_dtypes.float8_e4m3)),
    }
    maps = []
    for c, ci in enumerate(per_core_inputs):
        ci = dict(ci)
        ci["xown"] = np.ascontiguousarray(xp[c * npc:(c + 1) * npc])
        maps.append(dict(shared, **ci))
    return maps


# ------------------------------------------------------------------- kernel

def kernel(x, edge_index, W1, b1, W2, b2, _trace=False):
    from concourse.bass_utils import run_bass_kernel_spmd

    x = np.asarray(x)
    n_nodes = x.shape[0]
    meta, pci = host_prep(edge_index, n_nodes, N_CORES)
    nc = build_program(meta)
    in_maps = make_core_inputs(meta, pci, x, W1, b1, W2, b2)
    res = run_bass_kernel_spmd(nc, in_maps, list(range(N_CORES)),
                               trace=_trace)
    out = np.concatenate([res.results[c]["out"] for c in range(N_CORES)],
                         axis=0)
    if _trace:
        return out, res
    return out


# revision 35
# speedup vs baseline: 2.5922x; 1.4972x over previous
"""Two-layer GCN (MultiOrderGraphLayer) Bass kernel for 8 Trainium2 cores.

Math: out = 0.5*(relu(A_hat@x@W1+b1) + relu(A_hat@x@W2+b2)) with
A_hat = D^-1/2 (A+I) D^-1/2.  Both layers share A_hat, so g = A_hat @ x is
computed once; the two small 128x128 matmuls run afterwards.

Normalization is split: norm_e = dinv[src]*dinv[dst] factors, so
  - host prescales x' = dinv (.) x  (bf16, one rounding),
  - the per-edge one-hot is a PURE 0/1 pattern (exact in fp8e4m3,
    precomputed on host, fully preloaded to SBUF -- no DVE work, no
    mid-loop DMA),
  - dinv[dst] is applied in phase 2 as a per-partition ACT scale on the
    relu (nodes are partitions there); the bias matmul uses a
    sqrt(deg) row instead of ones so relu(dinv*(gW + sqrt(deg)*b)) =
    relu(dinv*gW + b) exactly.

Edge layout (per core): edges partitioned by destination core, split into
lo/hi source streams (int16 gather index limit), each stream sorted by
128-node destination window but blocked WITHOUT per-window padding -- only
a single tail pad per stream (1.01x instead of 1.21x slots).  Blocks may
span window boundaries; the shared program runs each block against every
window it touches on ANY core (union ranges), and each (window, block)
pair gets its own one-hot column pattern, zero outside the window, so
cores with different boundaries stay correct.

Per block: dma_gather 128 rows of x' (256B bf16) and matmul
t_T += xg^T @ S into PSUM over the window's blocks; finish:
out[n, fo] = relu(dinv[n]*(g_T^T @ (0.5*W) + sqrt(deg)*0.5*b)) summed
over the two layers, written node-major straight from PSUM matmuls.
"""

import math
import numpy as np

N_NODES = 50000
D = 128
N_CORES = 8
SPLIT = 32768  # int16 gather index limit
WIN = 128      # output-window size in nodes (one-hot width / psum free dim)
CHUNK = 2048   # indices per dma_gather instruction (multiple of 128);
               # must fit the per-queue SWDGE descriptor ring (scratch is
               # doubled below) so all 4 rings stay fed and drains overlap
N_QUEUES = 4   # SWDGE queues; rotating queue_num 4x's gather bandwidth
OH_LOADS = 4   # preamble DMAs that fill the one-hot SBUF tile
WGRP = 8       # windows per padding group (streams re-anchor at group ends)


# ---------------------------------------------------------------- host prep

def host_prep(edge_index, n_nodes, n_cores, split=SPLIT, chunk=CHUNK):
    """Preprocessing: edge partitioning, window-sorted lo/hi streams with
    tail-only padding, shared per-window block ranges, 0/1 one-hot pattern
    per (window, block) pair, per-node dinv."""
    import ml_dtypes

    src = np.asarray(edge_index[0], dtype=np.int64)
    dst = np.asarray(edge_index[1], dtype=np.int64)
    deg = np.bincount(dst, minlength=n_nodes).astype(np.int64) + 1
    dinv_node = (1.0 / np.sqrt(deg)).astype(np.float32)
    sqdeg_node = np.sqrt(deg).astype(np.float32)

    # self-loops are NOT gathered: each window adds its own contiguous
    # x' slice via one identity matmul instead (phase 2's dinv[dst]
    # scale turns dinv[d]*x[d] into the dinv^2 self-loop term).
    s_all, d_all = src, dst

    npc = n_nodes // n_cores
    assert npc * n_cores == n_nodes
    nwin = math.ceil(npc / WIN)
    n_halves = 2 if n_nodes > split else 1

    core_streams = []  # [c][h] -> (srcrel, dloc, win)
    offs_all = np.zeros((n_cores, n_halves, nwin + 1), np.int64)
    for c in range(n_cores):
        n0 = c * npc
        m = (d_all >= n0) & (d_all < n0 + npc)
        s, d = s_all[m], d_all[m]
        w = (d - n0) // WIN
        half = (s >= split).astype(np.int64) if n_halves == 2 else np.zeros_like(s)
        per_h = []
        for h in range(n_halves):
            sel = half == h
            sh, dh, wh = s[sel], d[sel], w[sel]
            order = np.argsort(wh, kind="stable")
            sh, dh, wh = sh[order], dh[order], wh[order]
            cnt = np.bincount(wh, minlength=nwin)
            offs_all[c, h, 1:] = np.cumsum(cnt)
            per_h.append((sh - h * split, dh - n0 - wh * WIN, wh))
        core_streams.append(per_h)

    # group-anchored padding: streams padded to a shared length at the end
    # of every WGRP-window group, so per-core prefix drift cannot
    # accumulate; per-window block ranges are unions over cores within
    # the group.
    n_grp = -(-nwin // WGRP)
    glen = np.zeros((n_halves, n_grp), np.int64)
    gbase = np.zeros((n_halves, n_grp + 1), np.int64)
    for h in range(n_halves):
        for g in range(n_grp):
            gs_w, ge_w = g * WGRP, min((g + 1) * WGRP, nwin)
            L = (offs_all[:, h, ge_w] - offs_all[:, h, gs_w]).max()
            glen[h, g] = -(-L // 128) * 128
        gbase[h, 1:] = np.cumsum(glen[h])
    shared_len = [int(gbase[h, -1]) for h in range(n_halves)]
    # anchored per-core offsets -> shared block ranges per (half, window)
    wstart = np.zeros((n_cores, n_halves, nwin), np.int64)
    wend = np.zeros((n_cores, n_halves, nwin), np.int64)
    for h in range(n_halves):
        for g in range(n_grp):
            gs_w, ge_w = g * WGRP, min((g + 1) * WGRP, nwin)
            for v in range(gs_w, ge_w):
                wstart[:, h, v] = gbase[h, g] + (offs_all[:, h, v]
                                                 - offs_all[:, h, gs_w])
                wend[:, h, v] = gbase[h, g] + (offs_all[:, h, v + 1]
                                               - offs_all[:, h, gs_w])
    bs = np.zeros((n_halves, nwin), np.int64)
    be = np.zeros((n_halves, nwin), np.int64)
    for h in range(n_halves):
        bs[h] = wstart[:, h, :].min(axis=0) // 128
        be[h] = np.maximum(-(-wend[:, h, :].max(axis=0) // 128), bs[h])
    seq_tot = int((be - bs).sum())

    # chunk split per half (shared across cores)
    chunk_sizes = []
    for h in range(n_halves):
        rem, sizes = shared_len[h], []
        while rem > 0:
            L = min(chunk, rem)
            sizes.append(L)
            rem -= L
        chunk_sizes.append(sizes)

    per_core_inputs = []
    lanes = np.arange(128)
    for c in range(n_cores):
        slot_src, slot_dl, slot_win = [], [], []
        for h in range(n_halves):
            sh, dlh, wh = core_streams[c][h]
            ss = np.zeros(shared_len[h], np.int64)
            sd = np.full(shared_len[h], -1, np.int64)
            sw = np.full(shared_len[h], -1, np.int64)
            for g in range(n_grp):
                gs_w, ge_w = g * WGRP, min((g + 1) * WGRP, nwin)
                a = offs_all[c, h, gs_w]
                b = offs_all[c, h, ge_w]
                gb = int(gbase[h, g])
                ss[gb:gb + (b - a)] = sh[a:b]
                sd[gb:gb + (b - a)] = dlh[a:b]
                sw[gb:gb + (b - a)] = wh[a:b]
            slot_src.append(ss)
            slot_dl.append(sd)
            slot_win.append(sw)

        # one-hot pattern per emission seq: for v: for h: for b in [bs, be)
        ohp = np.zeros((128, seq_tot, 128), dtype=ml_dtypes.float8_e4m3)
        seq = 0
        for v in range(nwin):
            for h in range(n_halves):
                for b in range(int(bs[h][v]), int(be[h][v])):
                    sl = slice(b * 128, (b + 1) * 128)
                    msk = slot_win[h][sl] == v
                    ohp[lanes[msk], seq, slot_dl[h][sl][msk]] = 1.0
                    seq += 1
        assert seq == seq_tot

        dv = np.ones((128, nwin), np.float32)
        for wi in range(nwin):
            wl = min(WIN, npc - wi * WIN)
            dv[:wl, wi] = dinv_node[c * npc + wi * WIN:c * npc + wi * WIN + wl]
        core_in = {
            "ohpat": np.ascontiguousarray(ohp.reshape(128, seq_tot * 128)),
            "dinvp": dv,
            "sqdeg": np.ascontiguousarray(
                sqdeg_node[c * npc:(c + 1) * npc].astype(ml_dtypes.bfloat16)
            ).reshape(1, npc),
        }
        # gather indices: wrapped [16, L/16] per chunk, replicated to 128 rows
        for h in range(n_halves):
            stream = slot_src[h].astype(np.int16)
            cols, off = [], 0
            for L in chunk_sizes[h]:
                a = stream[off:off + L].reshape(-1, 16).T
                cols.append(a)
                off += L
            wrapped = np.concatenate(cols, axis=1)
            core_in["idx_h%d" % h] = np.ascontiguousarray(np.tile(wrapped, (8, 1)))
        per_core_inputs.append(core_in)

    meta = dict(n_nodes=n_nodes, n_cores=n_cores, npc=npc, nwin=nwin,
                n_halves=n_halves, split=split, bs=bs, be=be,
                shared_len=shared_len, seq_tot=seq_tot, chunk=chunk,
                chunk_sizes=chunk_sizes, dinv_node=dinv_node)
    return meta, per_core_inputs


# ------------------------------------------------------------- bass program

def build_program(meta):
    import concourse.bacc as bacc
    import concourse.mybir as mybir
    import concourse.tile as tile
    from concourse import library_config

    f32 = mybir.dt.float32
    bf16 = mybir.dt.bfloat16
    fp8 = mybir.dt.float8e4
    i16 = mybir.dt.int16
    AF = mybir.ActivationFunctionType
    OP = mybir.AluOpType

    n_nodes = meta["n_nodes"]
    npc, nwin = meta["npc"], meta["nwin"]
    n_halves, split = meta["n_halves"], meta["split"]
    bs, be = meta["bs"], meta["be"]
    seq_tot = meta["seq_tot"]
    chunk = meta["chunk"]
    chunk_sizes = meta["chunk_sizes"]

    nc = bacc.Bacc("TRN2", num_swdge_queues=N_QUEUES,
                   dynamic_dma_scratch_size=32768)

    x_d = nc.declare_dram_parameter("x", [n_nodes, D], bf16, isOutput=False)
    oh_d = nc.declare_dram_parameter("ohpat", [128, seq_tot * 128], fp8,
                                     isOutput=False)
    dv_d = nc.declare_dram_parameter("dinvp", [128, nwin], f32, isOutput=False)
    sq_d = nc.declare_dram_parameter("sqdeg", [1, npc], bf16, isOutput=False)
    idx_d = [nc.declare_dram_parameter("idx_h%d" % h,
                                       [128, meta["shared_len"][h] // 16],
                                       i16, isOutput=False)
             for h in range(n_halves)]
    w1_d = nc.declare_dram_parameter("W1", [D, D], f32, isOutput=False)
    w2_d = nc.declare_dram_parameter("W2", [D, D], f32, isOutput=False)
    b1_d = nc.declare_dram_parameter("b1", [1, D], f32, isOutput=False)
    b2_d = nc.declare_dram_parameter("b2", [1, D], f32, isOutput=False)
    xo_d = nc.declare_dram_parameter("xown", [npc, D], bf16, isOutput=False)
    id_d = nc.declare_dram_parameter("ident", [128, 128], fp8, isOutput=False)
    out_d = nc.declare_dram_parameter("out", [npc, D], f32, isOutput=True)

    WG = 4  # windows per phase-2 batch (one 512-wide psum bank)

    with tile.TileContext(nc) as tc:
        with (
            tc.tile_pool(name="const", bufs=1) as constp,
            tc.tile_pool(name="xg", bufs=8) as xgp,
            tc.tile_pool(name="ps1", bufs=3, space="PSUM") as ps1,
            tc.tile_pool(name="ps2", bufs=2, space="PSUM") as ps2,
            tc.tile_pool(name="fin", bufs=2) as finp,
        ):
            # Q7 library holding DMAGatherAnt; must precede all gathers
            nc.gpsimd.load_library(library_config.mlp)

            # idx streams first on the sync ring so gathers start ASAP
            idx_all = []
            for h in range(n_halves):
                t = constp.tile([128, meta["shared_len"][h] // 16], i16,
                                tag="idx%d" % h)
                nc.sync.dma_start(t[:], idx_d[h][:])
                idx_all.append(t)

            # big one-hot / xown loads on the scalar ring, in parallel
            # with the sync ring's preamble
            oh_all = constp.tile([128, seq_tot, 128], fp8)
            oh_view = oh_d[:].rearrange("p (b n) -> p b n", n=128)
            step = -(-seq_tot // OH_LOADS)
            for a in range(0, seq_tot, step):
                z = min(seq_tot, a + step)
                nc.scalar.dma_start(oh_all[:, a:z, :], oh_view[:, a:z, :])
            xow = constp.tile([128, nwin, 128], bf16)
            nfull = npc // 128
            nc.scalar.dma_start(
                xow[:, :nfull, :],
                xo_d[0:nfull * 128, :].rearrange("(w p) f -> p w f", p=128))
            if npc % 128:
                nc.scalar.dma_start(xow[: npc % 128, nfull, :],
                                    xo_d[nfull * 128:npc, :])

            # --- constants / metadata
            wts = {}
            for nm, src_d in (("w1", w1_d), ("w2", w2_d)):
                raw = constp.tile([128, 128], f32, tag=nm + "raw")
                nc.sync.dma_start(raw[:], src_d[:])
                half = constp.tile([128, 128], bf16, tag=nm + "half")
                nc.scalar.activation(half[:], raw[:], AF.Copy, scale=0.5)
                wts[nm] = half
            bias = {}
            for nm, src_d in (("b1", b1_d), ("b2", b2_d)):
                raw = constp.tile([1, 128], f32, tag=nm + "raw")
                nc.sync.dma_start(raw[:], src_d[:])
                half = constp.tile([1, 128], bf16, tag=nm + "half")
                nc.scalar.activation(half[:], raw[:], AF.Copy, scale=0.5)
                bias[nm] = half
            dinvp = constp.tile([128, nwin], f32)
            nc.sync.dma_start(dinvp[:], dv_d[:])
            sqdeg = constp.tile([1, npc], bf16)
            nc.sync.dma_start(sqdeg[:], sq_d[:])
            ident = constp.tile([128, 128], fp8)
            nc.sync.dma_start(ident[:], id_d[:])

            g_all = constp.tile([128, npc], bf16)

            # per-half stream state: lazy chunk issuing in window order
            class Stream:
                pass

            streams = []
            for h in range(n_halves):
                s = Stream()
                s.h = h
                s.base = x_d[0:split, :] if h == 0 else x_d[split:n_nodes, :]
                s.chunk_bounds = []
                off = 0
                for L in chunk_sizes[h]:
                    s.chunk_bounds.append((off, L))
                    off += L
                s.blk2chunk = np.repeat(
                    np.arange(len(chunk_sizes[h])),
                    [L // 128 for L in chunk_sizes[h]])
                s.tiles = {}
                streams.append(s)

            ci_global = 0

            def ensure_chunk(s, ci):
                nonlocal ci_global
                if ci in s.tiles:
                    return s.tiles[ci]
                off, L = s.chunk_bounds[ci]
                xg = xgp.tile([128, chunk // 128, 128], bf16, tag="xg")
                nc.gpsimd.dma_gather(
                    out_ap=xg[:, : L // 128, :],
                    in_ap=s.base,
                    idxs_ap=idx_all[s.h][:, off // 16:(off + L) // 16],
                    num_idxs=L,
                    num_idxs_reg=L,
                    elem_size=D,
                    single_packet=False,
                    queue_num=ci_global % N_QUEUES,
                )
                ci_global += 1
                # keep two chunks cached: windows whose block range
                # straddles a chunk boundary revisit the previous chunk
                while len(s.tiles) >= 2:
                    del s.tiles[min(s.tiles)]
                s.tiles[ci] = xg
                return xg

            # --- output stage: every WG windows, two weight matmuls +
            # per-window dinv-scaled relu + combine; overlaps aggregation
            def emit_phase2(wlo, whi):
                nwg = whi - wlo + 1
                wls = [min(WIN, npc - w * WIN) for w in range(wlo, whi + 1)]
                pps = {}
                for nm_w, nm_b in (("w1", "b1"), ("w2", "b2")):
                    pp = ps2.tile([128, WG * 128], f32, tag="pp")
                    for j, w in enumerate(range(wlo, whi + 1)):
                        wl = wls[j]
                        sl = pp[:wl, j * 128:(j + 1) * 128]
                        nc.tensor.matmul(sl, g_all[:, w * WIN:w * WIN + wl],
                                         wts[nm_w][:], start=True, stop=False)
                        nc.tensor.matmul(sl, sqdeg[:, w * WIN:w * WIN + wl],
                                         bias[nm_b][:], start=False, stop=True)
                    o = finp.tile([128, WG, 128], f32, tag="o" + nm_w)
                    for j, w in enumerate(range(wlo, whi + 1)):
                        nc.scalar.activation(
                            o[:wls[j], j, :],
                            pp[:wls[j], j * 128:(j + 1) * 128], AF.Relu,
                            scale=dinvp[:wls[j], w:w + 1])
                    pps[nm_w] = o
                ot = finp.tile([128, WG, 128], f32, tag="ot")
                otf = ot[:].rearrange("p c n -> p (c n)")
                o1f = pps["w1"][:].rearrange("p c n -> p (c n)")
                o2f = pps["w2"][:].rearrange("p c n -> p (c n)")
                rows = min(wls)
                if rows == 128:
                    nc.vector.tensor_tensor(otf[:, :nwg * 128],
                                            o1f[:, :nwg * 128],
                                            o2f[:, :nwg * 128], op=OP.add)
                else:
                    for j in range(nwg):
                        cs = slice(j * 128, j * 128 + 128)
                        nc.vector.tensor_tensor(otf[:wls[j], cs],
                                                o1f[:wls[j], cs],
                                                o2f[:wls[j], cs], op=OP.add)
                for j, w in enumerate(range(wlo, whi + 1)):
                    nc.sync.dma_start(out_d[w * WIN:w * WIN + wls[j], :],
                                      ot[:wls[j], j, :])

            seq = 0
            for w in range(nwin):
                wlen = min(WIN, npc - w * WIN)
                pw = ps1.tile([128, 128], f32, tag="pw")
                n_tot = 1 + sum(int(be[s.h][w] - bs[s.h][w]) for s in streams)
                # self-loop: pw += xown_w^T @ I  (exact fp8 identity)
                nc.tensor.matmul(pw[:, :wlen], xow[:wlen, w, :],
                                 ident[:wlen, :wlen],
                                 start=True, stop=(n_tot == 1))
                k = 1
                for s in streams:
                    for b in range(int(bs[s.h][w]), int(be[s.h][w])):
                        ci = int(s.blk2chunk[b])
                        xg = ensure_chunk(s, ci)
                        bl = (b * 128 - s.chunk_bounds[ci][0]) // 128
                        nc.tensor.matmul(
                            pw[:, :wlen],
                            xg[:, bl, :],
                            oh_all[:, seq, :wlen],
                            start=False,
                            stop=(k == n_tot - 1),
                        )
                        seq += 1
                        k += 1
                nc.scalar.activation(g_all[:, w * WIN:w * WIN + wlen],
                                     pw[:, :wlen], AF.Copy)
                if w % WG == WG - 1 or w == nwin - 1:
                    emit_phase2(w - (w % WG), w)
            assert seq == seq_tot

    nc.compile()
    return nc


def make_core_inputs(meta, per_core_inputs, x, W1, b1, W2, b2):
    """Full in_maps for run_bass_kernel_spmd (adds shared tensors)."""
    import ml_dtypes
    xf = np.asarray(x, np.float32)
    dinv = meta["dinv_node"]
    xp = (xf * dinv[:, None]).astype(ml_dtypes.bfloat16)
    npc = meta["npc"]
    shared = {
        "x": np.ascontiguousarray(xp),
        "W1": np.ascontiguousarray(np.asarray(W1, np.float32)),
        "W2": np.ascontiguousarray(np.asarray(W2, np.float32)),
        "b1": np.asarray(b1, np.float32).reshape(1, D),
        "b2": np.asarray(b2, np.float32).reshape(1, D),
        "ident": np.ascontiguousarray(
            np.eye(128, dtype=ml_dtypes.float8_e4m3)),
    }
    maps = []
    for c, ci in enumerate(per_core_inputs):
        ci = dict(ci)
        ci["xown"] = np.ascontiguousarray(xp[c * npc:(c + 1) * npc])
        maps.append(dict(shared, **ci))
    return maps


# ------------------------------------------------------------------- kernel

def kernel(x, edge_index, W1, b1, W2, b2, _trace=False):
    from concourse.bass_utils import run_bass_kernel_spmd

    x = np.asarray(x)
    n_nodes = x.shape[0]
    meta, pci = host_prep(edge_index, n_nodes, N_CORES)
    nc = build_program(meta)
    in_maps = make_core_inputs(meta, pci, x, W1, b1, W2, b2)
    res = run_bass_kernel_spmd(nc, in_maps, list(range(N_CORES)),
                               trace=_trace)
    out = np.concatenate([res.results[c]["out"] for c in range(N_CORES)],
                         axis=0)
    if _trace:
        return out, res
    return out


# revision 38
# speedup vs baseline: 2.7236x; 1.0507x over previous
"""Two-layer GCN (MultiOrderGraphLayer) Bass kernel for 8 Trainium2 cores.

Math: out = 0.5*(relu(A_hat@x@W1+b1) + relu(A_hat@x@W2+b2)) with
A_hat = D^-1/2 (A+I) D^-1/2.  Both layers share A_hat, so g = A_hat @ x is
computed once; the two small 128x128 matmuls run afterwards.

Normalization is split: norm_e = dinv[src]*dinv[dst] factors, so
  - host prescales x' = dinv (.) x  (bf16, one rounding),
  - the per-edge one-hot is a PURE 0/1 pattern (exact in fp8e4m3,
    precomputed on host, fully preloaded to SBUF -- no DVE work, no
    mid-loop DMA),
  - dinv[dst] is applied in phase 2 as a per-partition ACT scale on the
    relu (nodes are partitions there); the bias matmul uses a
    sqrt(deg) row instead of ones so relu(dinv*(gW + sqrt(deg)*b)) =
    relu(dinv*gW + b) exactly.

Edge layout (per core): edges partitioned by destination core, split into
lo/hi source streams (int16 gather index limit), each stream sorted by
128-node destination window but blocked WITHOUT per-window padding -- only
a single tail pad per stream (1.01x instead of 1.21x slots).  Blocks may
span window boundaries; the shared program runs each block against every
window it touches on ANY core (union ranges), and each (window, block)
pair gets its own one-hot column pattern, zero outside the window, so
cores with different boundaries stay correct.

Per block: dma_gather 128 rows of x' (256B bf16) and matmul
t_T += xg^T @ S into PSUM over the window's blocks; finish:
out[n, fo] = relu(dinv[n]*(g_T^T @ (0.5*W) + sqrt(deg)*0.5*b)) summed
over the two layers, written node-major straight from PSUM matmuls.
"""

import math
import numpy as np

N_NODES = 50000
D = 128
N_CORES = 8
SPLIT = 32768  # int16 gather index limit
WIN = 128      # output-window size in nodes (one-hot width / psum free dim)
CHUNK = 2048   # indices per dma_gather instruction (multiple of 128);
               # must fit the per-queue SWDGE descriptor ring (scratch is
               # doubled below) so all 4 rings stay fed and drains overlap
N_QUEUES = 4   # SWDGE queues; rotating queue_num 4x's gather bandwidth
OH_LOADS = 6   # preamble DMAs that fill the one-hot SBUF tile
WGRP = 8       # windows per padding group (streams re-anchor at group ends)


# ---------------------------------------------------------------- host prep

def host_prep(edge_index, n_nodes, n_cores, split=SPLIT, chunk=CHUNK):
    """Preprocessing: edge partitioning, window-sorted lo/hi streams with
    tail-only padding, shared per-window block ranges, 0/1 one-hot pattern
    per (window, block) pair, per-node dinv."""
    import ml_dtypes

    src = np.asarray(edge_index[0], dtype=np.int64)
    dst = np.asarray(edge_index[1], dtype=np.int64)
    deg = np.bincount(dst, minlength=n_nodes).astype(np.int64) + 1
    dinv_node = (1.0 / np.sqrt(deg)).astype(np.float32)
    sqdeg_node = np.sqrt(deg).astype(np.float32)

    # self-loops are NOT gathered: each window adds its own contiguous
    # x' slice via one identity matmul instead (phase 2's dinv[dst]
    # scale turns dinv[d]*x[d] into the dinv^2 self-loop term).
    s_all, d_all = src, dst

    npc = n_nodes // n_cores
    assert npc * n_cores == n_nodes
    nwin = math.ceil(npc / WIN)
    n_halves = 2 if n_nodes > split else 1

    core_streams = []  # [c][h] -> (srcrel, dloc, win)
    offs_all = np.zeros((n_cores, n_halves, nwin + 1), np.int64)
    for c in range(n_cores):
        n0 = c * npc
        m = (d_all >= n0) & (d_all < n0 + npc)
        s, d = s_all[m], d_all[m]
        w = (d - n0) // WIN
        half = (s >= split).astype(np.int64) if n_halves == 2 else np.zeros_like(s)
        per_h = []
        for h in range(n_halves):
            sel = half == h
            sh, dh, wh = s[sel], d[sel], w[sel]
            order = np.argsort(wh, kind="stable")
            sh, dh, wh = sh[order], dh[order], wh[order]
            cnt = np.bincount(wh, minlength=nwin)
            offs_all[c, h, 1:] = np.cumsum(cnt)
            per_h.append((sh - h * split, dh - n0 - wh * WIN, wh))
        core_streams.append(per_h)

    # group-anchored padding: streams padded to a shared length at the end
    # of every WGRP-window group, so per-core prefix drift cannot
    # accumulate; per-window block ranges are unions over cores within
    # the group.
    n_grp = -(-nwin // WGRP)
    glen = np.zeros((n_halves, n_grp), np.int64)
    gbase = np.zeros((n_halves, n_grp + 1), np.int64)
    for h in range(n_halves):
        for g in range(n_grp):
            gs_w, ge_w = g * WGRP, min((g + 1) * WGRP, nwin)
            L = (offs_all[:, h, ge_w] - offs_all[:, h, gs_w]).max()
            glen[h, g] = -(-L // 128) * 128
        gbase[h, 1:] = np.cumsum(glen[h])
    shared_len = [int(gbase[h, -1]) for h in range(n_halves)]
    # anchored per-core offsets -> shared block ranges per (half, window)
    wstart = np.zeros((n_cores, n_halves, nwin), np.int64)
    wend = np.zeros((n_cores, n_halves, nwin), np.int64)
    for h in range(n_halves):
        for g in range(n_grp):
            gs_w, ge_w = g * WGRP, min((g + 1) * WGRP, nwin)
            for v in range(gs_w, ge_w):
                wstart[:, h, v] = gbase[h, g] + (offs_all[:, h, v]
                                                 - offs_all[:, h, gs_w])
                wend[:, h, v] = gbase[h, g] + (offs_all[:, h, v + 1]
                                               - offs_all[:, h, gs_w])
    bs = np.zeros((n_halves, nwin), np.int64)
    be = np.zeros((n_halves, nwin), np.int64)
    for h in range(n_halves):
        bs[h] = wstart[:, h, :].min(axis=0) // 128
        be[h] = np.maximum(-(-wend[:, h, :].max(axis=0) // 128), bs[h])
    seq_tot = int((be - bs).sum())

    # chunk split per half (shared across cores)
    chunk_sizes = []
    for h in range(n_halves):
        rem, sizes = shared_len[h], []
        while rem > 0:
            L = min(chunk, rem)
            sizes.append(L)
            rem -= L
        chunk_sizes.append(sizes)

    per_core_inputs = []
    lanes = np.arange(128)
    for c in range(n_cores):
        slot_src, slot_dl, slot_win = [], [], []
        for h in range(n_halves):
            sh, dlh, wh = core_streams[c][h]
            ss = np.zeros(shared_len[h], np.int64)
            sd = np.full(shared_len[h], -1, np.int64)
            sw = np.full(shared_len[h], -1, np.int64)
            for g in range(n_grp):
                gs_w, ge_w = g * WGRP, min((g + 1) * WGRP, nwin)
                a = offs_all[c, h, gs_w]
                b = offs_all[c, h, ge_w]
                gb = int(gbase[h, g])
                ss[gb:gb + (b - a)] = sh[a:b]
                sd[gb:gb + (b - a)] = dlh[a:b]
                sw[gb:gb + (b - a)] = wh[a:b]
            slot_src.append(ss)
            slot_dl.append(sd)
            slot_win.append(sw)

        # one-hot pattern per emission seq: for v: for h: for b in [bs, be)
        ohp = np.zeros((128, seq_tot, 128), dtype=ml_dtypes.float8_e4m3)
        seq = 0
        for v in range(nwin):
            for h in range(n_halves):
                for b in range(int(bs[h][v]), int(be[h][v])):
                    sl = slice(b * 128, (b + 1) * 128)
                    msk = slot_win[h][sl] == v
                    ohp[lanes[msk], seq, slot_dl[h][sl][msk]] = 1.0
                    seq += 1
        assert seq == seq_tot

        dv = np.ones((128, nwin), np.float32)
        for wi in range(nwin):
            wl = min(WIN, npc - wi * WIN)
            dv[:wl, wi] = dinv_node[c * npc + wi * WIN:c * npc + wi * WIN + wl]
        core_in = {
            "ohpat": np.ascontiguousarray(ohp.reshape(128, seq_tot * 128)),
            "dinvp": dv,
            "sqdeg": np.ascontiguousarray(
                sqdeg_node[c * npc:(c + 1) * npc].astype(ml_dtypes.bfloat16)
            ).reshape(1, npc),
        }
        # gather indices: wrapped [16, L/16] per chunk, replicated to 128 rows
        for h in range(n_halves):
            stream = slot_src[h].astype(np.int16)
            cols, off = [], 0
            for L in chunk_sizes[h]:
                a = stream[off:off + L].reshape(-1, 16).T
                cols.append(a)
                off += L
            wrapped = np.concatenate(cols, axis=1)
            core_in["idx_h%d" % h] = np.ascontiguousarray(np.tile(wrapped, (8, 1)))
        per_core_inputs.append(core_in)

    meta = dict(n_nodes=n_nodes, n_cores=n_cores, npc=npc, nwin=nwin,
                n_halves=n_halves, split=split, bs=bs, be=be,
                shared_len=shared_len, seq_tot=seq_tot, chunk=chunk,
                chunk_sizes=chunk_sizes, dinv_node=dinv_node)
    return meta, per_core_inputs


# ------------------------------------------------------------- bass program

def build_program(meta):
    import concourse.bacc as bacc
    import concourse.mybir as mybir
    import concourse.tile as tile
    from concourse import library_config

    f32 = mybir.dt.float32
    bf16 = mybir.dt.bfloat16
    fp8 = mybir.dt.float8e4
    i16 = mybir.dt.int16
    AF = mybir.ActivationFunctionType
    OP = mybir.AluOpType

    n_nodes = meta["n_nodes"]
    npc, nwin = meta["npc"], meta["nwin"]
    n_halves, split = meta["n_halves"], meta["split"]
    bs, be = meta["bs"], meta["be"]
    seq_tot = meta["seq_tot"]
    chunk = meta["chunk"]
    chunk_sizes = meta["chunk_sizes"]

    nc = bacc.Bacc("TRN2", num_swdge_queues=N_QUEUES,
                   dynamic_dma_scratch_size=32768)

    x_d = nc.declare_dram_parameter("x", [n_nodes, D], bf16, isOutput=False)
    oh_d = nc.declare_dram_parameter("ohpat", [128, seq_tot * 128], fp8,
                                     isOutput=False)
    dv_d = nc.declare_dram_parameter("dinvp", [128, nwin], f32, isOutput=False)
    sq_d = nc.declare_dram_parameter("sqdeg", [1, npc], bf16, isOutput=False)
    idx_d = [nc.declare_dram_parameter("idx_h%d" % h,
                                       [128, meta["shared_len"][h] // 16],
                                       i16, isOutput=False)
             for h in range(n_halves)]
    w1_d = nc.declare_dram_parameter("W1", [D, D], f32, isOutput=False)
    w2_d = nc.declare_dram_parameter("W2", [D, D], f32, isOutput=False)
    b1_d = nc.declare_dram_parameter("b1", [1, D], f32, isOutput=False)
    b2_d = nc.declare_dram_parameter("b2", [1, D], f32, isOutput=False)
    xo_d = nc.declare_dram_parameter("xown", [npc, D], bf16, isOutput=False)
    id_d = nc.declare_dram_parameter("ident", [128, 128], fp8, isOutput=False)
    out_d = nc.declare_dram_parameter("out", [npc, D], f32, isOutput=True)

    WG = 4  # windows per phase-2 batch (one 512-wide psum bank)

    with tile.TileContext(nc) as tc:
        with (
            tc.tile_pool(name="const", bufs=1) as constp,
            tc.tile_pool(name="xg", bufs=8) as xgp,
            tc.tile_pool(name="ps1", bufs=3, space="PSUM") as ps1,
            tc.tile_pool(name="ps2", bufs=2, space="PSUM") as ps2,
            tc.tile_pool(name="fin", bufs=2) as finp,
        ):
            # Q7 library holding DMAGatherAnt; must precede all gathers
            nc.gpsimd.load_library(library_config.mlp)

            # idx streams first on the sync ring so gathers start ASAP
            idx_all = []
            for h in range(n_halves):
                t = constp.tile([128, meta["shared_len"][h] // 16], i16,
                                tag="idx%d" % h)
                nc.sync.dma_start(t[:], idx_d[h][:])
                idx_all.append(t)

            # xown first (small, needed by every window close), then the
            # big one-hot pieces -- all on the scalar ring, in parallel
            # with the sync ring's preamble
            xow = constp.tile([128, nwin, 128], bf16)
            nfull = npc // 128
            nc.scalar.dma_start(
                xow[:, :nfull, :],
                xo_d[0:nfull * 128, :].rearrange("(w p) f -> p w f", p=128))
            if npc % 128:
                nc.scalar.dma_start(xow[: npc % 128, nfull, :],
                                    xo_d[nfull * 128:npc, :])
            oh_all = constp.tile([128, seq_tot, 128], fp8)
            oh_view = oh_d[:].rearrange("p (b n) -> p b n", n=128)
            step = -(-seq_tot // OH_LOADS)
            for a in range(0, seq_tot, step):
                z = min(seq_tot, a + step)
                nc.scalar.dma_start(oh_all[:, a:z, :], oh_view[:, a:z, :])

            # --- constants / metadata
            wts = {}
            for nm, src_d in (("w1", w1_d), ("w2", w2_d)):
                raw = constp.tile([128, 128], f32, tag=nm + "raw")
                nc.sync.dma_start(raw[:], src_d[:])
                half = constp.tile([128, 128], bf16, tag=nm + "half")
                nc.scalar.activation(half[:], raw[:], AF.Copy, scale=0.5)
                wts[nm] = half
            bias = {}
            for nm, src_d in (("b1", b1_d), ("b2", b2_d)):
                raw = constp.tile([1, 128], f32, tag=nm + "raw")
                nc.sync.dma_start(raw[:], src_d[:])
                half = constp.tile([1, 128], bf16, tag=nm + "half")
                nc.scalar.activation(half[:], raw[:], AF.Copy, scale=0.5)
                bias[nm] = half
            dinvp = constp.tile([128, nwin], f32)
            nc.sync.dma_start(dinvp[:], dv_d[:])
            sqdeg = constp.tile([1, npc], bf16)
            nc.sync.dma_start(sqdeg[:], sq_d[:])
            ident = constp.tile([128, 128], fp8)
            nc.sync.dma_start(ident[:], id_d[:])

            g_all = constp.tile([128, npc], bf16)

            # per-half stream state: lazy chunk issuing in window order
            class Stream:
                pass

            streams = []
            for h in range(n_halves):
                s = Stream()
                s.h = h
                s.base = x_d[0:split, :] if h == 0 else x_d[split:n_nodes, :]
                s.chunk_bounds = []
                off = 0
                for L in chunk_sizes[h]:
                    s.chunk_bounds.append((off, L))
                    off += L
                s.blk2chunk = np.repeat(
                    np.arange(len(chunk_sizes[h])),
                    [L // 128 for L in chunk_sizes[h]])
                s.tiles = {}
                streams.append(s)

            ci_global = 0

            def ensure_chunk(s, ci):
                nonlocal ci_global
                if ci in s.tiles:
                    return s.tiles[ci]
                off, L = s.chunk_bounds[ci]
                xg = xgp.tile([128, chunk // 128, 128], bf16, tag="xg")
                nc.gpsimd.dma_gather(
                    out_ap=xg[:, : L // 128, :],
                    in_ap=s.base,
                    idxs_ap=idx_all[s.h][:, off // 16:(off + L) // 16],
                    num_idxs=L,
                    num_idxs_reg=L,
                    elem_size=D,
                    single_packet=False,
                    queue_num=ci_global % N_QUEUES,
                )
                ci_global += 1
                # keep two chunks cached: windows whose block range
                # straddles a chunk boundary revisit the previous chunk
                while len(s.tiles) >= 2:
                    del s.tiles[min(s.tiles)]
                s.tiles[ci] = xg
                return xg

            # --- output stage: every WG windows, two weight matmuls +
            # per-window dinv-scaled relu + combine; overlaps aggregation
            def emit_phase2(wlo, whi):
                nwg = whi - wlo + 1
                wls = [min(WIN, npc - w * WIN) for w in range(wlo, whi + 1)]
                pps = {}
                for nm_w, nm_b in (("w1", "b1"), ("w2", "b2")):
                    pp = ps2.tile([128, WG * 128], f32, tag="pp")
                    for j, w in enumerate(range(wlo, whi + 1)):
                        wl = wls[j]
                        sl = pp[:wl, j * 128:(j + 1) * 128]
                        nc.tensor.matmul(sl, g_all[:, w * WIN:w * WIN + wl],
                                         wts[nm_w][:], start=True, stop=False)
                        nc.tensor.matmul(sl, sqdeg[:, w * WIN:w * WIN + wl],
                                         bias[nm_b][:], start=False, stop=True)
                    o = finp.tile([128, WG, 128], f32, tag="o" + nm_w)
                    for j, w in enumerate(range(wlo, whi + 1)):
                        nc.scalar.activation(
                            o[:wls[j], j, :],
                            pp[:wls[j], j * 128:(j + 1) * 128], AF.Relu,
                            scale=dinvp[:wls[j], w:w + 1])
                    pps[nm_w] = o
                ot = finp.tile([128, WG, 128], f32, tag="ot")
                otf = ot[:].rearrange("p c n -> p (c n)")
                o1f = pps["w1"][:].rearrange("p c n -> p (c n)")
                o2f = pps["w2"][:].rearrange("p c n -> p (c n)")
                rows = min(wls)
                if rows == 128:
                    nc.vector.tensor_tensor(otf[:, :nwg * 128],
                                            o1f[:, :nwg * 128],
                                            o2f[:, :nwg * 128], op=OP.add)
                else:
                    for j in range(nwg):
                        cs = slice(j * 128, j * 128 + 128)
                        nc.vector.tensor_tensor(otf[:wls[j], cs],
                                                o1f[:wls[j], cs],
                                                o2f[:wls[j], cs], op=OP.add)
                for j, w in enumerate(range(wlo, whi + 1)):
                    nc.sync.dma_start(out_d[w * WIN:w * WIN + wls[j], :],
                                      ot[:wls[j], j, :])

            seq = 0
            for w in range(nwin):
                wlen = min(WIN, npc - w * WIN)
                pw = ps1.tile([128, 128], f32, tag="pw")
                n_tot = 1 + sum(int(be[s.h][w] - bs[s.h][w]) for s in streams)
                k = 0
                for s in streams:
                    for b in range(int(bs[s.h][w]), int(be[s.h][w])):
                        ci = int(s.blk2chunk[b])
                        xg = ensure_chunk(s, ci)
                        bl = (b * 128 - s.chunk_bounds[ci][0]) // 128
                        nc.tensor.matmul(
                            pw[:, :wlen],
                            xg[:, bl, :],
                            oh_all[:, seq, :wlen],
                            start=(k == 0),
                            stop=False,
                        )
                        seq += 1
                        k += 1
                # self-loop last: pw += xown_w^T @ I  (exact fp8 identity);
                # placed at the close so edge matmuls never wait on xow
                nc.tensor.matmul(pw[:, :wlen], xow[:wlen, w, :],
                                 ident[:wlen, :wlen],
                                 start=(k == 0), stop=True)
                nc.scalar.activation(g_all[:, w * WIN:w * WIN + wlen],
                                     pw[:, :wlen], AF.Copy)
                if w % WG == WG - 1 or w == nwin - 1:
                    emit_phase2(w - (w % WG), w)
            assert seq == seq_tot

    nc.compile()
    return nc


def make_core_inputs(meta, per_core_inputs, x, W1, b1, W2, b2):
    """Full in_maps for run_bass_kernel_spmd (adds shared tensors)."""
    import ml_dtypes
    xf = np.asarray(x, np.float32)
    dinv = meta["dinv_node"]
    xp = (xf * dinv[:, None]).astype(ml_dtypes.bfloat16)
    npc = meta["npc"]
    shared = {
        "x": np.ascontiguousarray(xp),
        "W1": np.ascontiguousarray(np.asarray(W1, np.float32)),
        "W2": np.ascontiguousarray(np.asarray(W2, np.float32)),
        "b1": np.asarray(b1, np.float32).reshape(1, D),
        "b2": np.asarray(b2, np.float32).reshape(1, D),
        "ident": np.ascontiguousarray(
            np.eye(128, dtype=ml_dtypes.float8_e4m3)),
    }
    maps = []
    for c, ci in enumerate(per_core_inputs):
        ci = dict(ci)
        ci["xown"] = np.ascontiguousarray(xp[c * npc:(c + 1) * npc])
        maps.append(dict(shared, **ci))
    return maps


# ------------------------------------------------------------------- kernel

def kernel(x, edge_index, W1, b1, W2, b2, _trace=False):
    from concourse.bass_utils import run_bass_kernel_spmd

    x = np.asarray(x)
    n_nodes = x.shape[0]
    meta, pci = host_prep(edge_index, n_nodes, N_CORES)
    nc = build_program(meta)
    in_maps = make_core_inputs(meta, pci, x, W1, b1, W2, b2)
    res = run_bass_kernel_spmd(nc, in_maps, list(range(N_CORES)),
                               trace=_trace)
    out = np.concatenate([res.results[c]["out"] for c in range(N_CORES)],
                         axis=0)
    if _trace:
        return out, res
    return out


# revision 41
# speedup vs baseline: 2.7613x; 1.0138x over previous
"""Two-layer GCN (MultiOrderGraphLayer) Bass kernel for 8 Trainium2 cores.

Math: out = 0.5*(relu(A_hat@x@W1+b1) + relu(A_hat@x@W2+b2)) with
A_hat = D^-1/2 (A+I) D^-1/2.  Both layers share A_hat, so g = A_hat @ x is
computed once; the two small 128x128 matmuls run afterwards.

Normalization is split: norm_e = dinv[src]*dinv[dst] factors, so
  - host prescales x' = dinv (.) x  (bf16, one rounding),
  - the per-edge one-hot is a PURE 0/1 pattern (exact in fp8e4m3,
    precomputed on host, fully preloaded to SBUF -- no DVE work, no
    mid-loop DMA),
  - dinv[dst] is applied in phase 2 as a per-partition ACT scale on the
    relu (nodes are partitions there); the bias matmul uses a
    sqrt(deg) row instead of ones so relu(dinv*(gW + sqrt(deg)*b)) =
    relu(dinv*gW + b) exactly.

Edge layout (per core): edges partitioned by destination core, split into
lo/hi source streams (int16 gather index limit), each stream sorted by
128-node destination window but blocked WITHOUT per-window padding -- only
a single tail pad per stream (1.01x instead of 1.21x slots).  Blocks may
span window boundaries; the shared program runs each block against every
window it touches on ANY core (union ranges), and each (window, block)
pair gets its own one-hot column pattern, zero outside the window, so
cores with different boundaries stay correct.

Per block: dma_gather 128 rows of x' (256B bf16) and matmul
t_T += xg^T @ S into PSUM over the window's blocks; finish:
out[n, fo] = relu(dinv[n]*(g_T^T @ (0.5*W) + sqrt(deg)*0.5*b)) summed
over the two layers, written node-major straight from PSUM matmuls.
"""

import math
import numpy as np

N_NODES = 50000
D = 128
N_CORES = 8
SPLIT = 32768  # int16 gather index limit
WIN = 128      # output-window size in nodes (one-hot width / psum free dim)
CHUNK = 2048   # indices per dma_gather instruction (multiple of 128);
               # must fit the per-queue SWDGE descriptor ring (scratch is
               # doubled below) so all 4 rings stay fed and drains overlap
N_QUEUES = 4   # SWDGE queues; rotating queue_num 4x's gather bandwidth
OH_LOADS = 6   # preamble DMAs that fill the one-hot SBUF tile
WGRP = 8       # windows per padding group (streams re-anchor at group ends)


# ---------------------------------------------------------------- host prep

def host_prep(edge_index, n_nodes, n_cores, split=SPLIT, chunk=CHUNK):
    """Preprocessing: edge partitioning, window-sorted lo/hi streams with
    tail-only padding, shared per-window block ranges, 0/1 one-hot pattern
    per (window, block) pair, per-node dinv."""
    import ml_dtypes

    src = np.asarray(edge_index[0], dtype=np.int64)
    dst = np.asarray(edge_index[1], dtype=np.int64)
    deg = np.bincount(dst, minlength=n_nodes).astype(np.int64) + 1
    dinv_node = (1.0 / np.sqrt(deg)).astype(np.float32)
    sqdeg_node = np.sqrt(deg).astype(np.float32)

    # self-loops are NOT gathered: each window adds its own contiguous
    # x' slice via one identity matmul instead (phase 2's dinv[dst]
    # scale turns dinv[d]*x[d] into the dinv^2 self-loop term).
    s_all, d_all = src, dst

    npc = n_nodes // n_cores
    assert npc * n_cores == n_nodes
    nwin = math.ceil(npc / WIN)
    n_halves = 2 if n_nodes > split else 1

    core_streams = []  # [c][h] -> (srcrel, dloc, win)
    offs_all = np.zeros((n_cores, n_halves, nwin + 1), np.int64)
    for c in range(n_cores):
        n0 = c * npc
        m = (d_all >= n0) & (d_all < n0 + npc)
        s, d = s_all[m], d_all[m]
        w = (d - n0) // WIN
        half = (s >= split).astype(np.int64) if n_halves == 2 else np.zeros_like(s)
        per_h = []
        for h in range(n_halves):
            sel = half == h
            sh, dh, wh = s[sel], d[sel], w[sel]
            order = np.argsort(wh, kind="stable")
            sh, dh, wh = sh[order], dh[order], wh[order]
            cnt = np.bincount(wh, minlength=nwin)
            offs_all[c, h, 1:] = np.cumsum(cnt)
            per_h.append((sh - h * split, dh - n0 - wh * WIN, wh))
        core_streams.append(per_h)

    # group-anchored padding: streams padded to a shared length at the end
    # of every WGRP-window group, so per-core prefix drift cannot
    # accumulate; per-window block ranges are unions over cores within
    # the group.
    n_grp = -(-nwin // WGRP)
    glen = np.zeros((n_halves, n_grp), np.int64)
    gbase = np.zeros((n_halves, n_grp + 1), np.int64)
    for h in range(n_halves):
        for g in range(n_grp):
            gs_w, ge_w = g * WGRP, min((g + 1) * WGRP, nwin)
            L = (offs_all[:, h, ge_w] - offs_all[:, h, gs_w]).max()
            glen[h, g] = -(-L // 128) * 128
        gbase[h, 1:] = np.cumsum(glen[h])
    shared_len = [int(gbase[h, -1]) for h in range(n_halves)]
    # anchored per-core offsets -> shared block ranges per (half, window)
    wstart = np.zeros((n_cores, n_halves, nwin), np.int64)
    wend = np.zeros((n_cores, n_halves, nwin), np.int64)
    for h in range(n_halves):
        for g in range(n_grp):
            gs_w, ge_w = g * WGRP, min((g + 1) * WGRP, nwin)
            for v in range(gs_w, ge_w):
                wstart[:, h, v] = gbase[h, g] + (offs_all[:, h, v]
                                                 - offs_all[:, h, gs_w])
                wend[:, h, v] = gbase[h, g] + (offs_all[:, h, v + 1]
                                               - offs_all[:, h, gs_w])
    bs = np.zeros((n_halves, nwin), np.int64)
    be = np.zeros((n_halves, nwin), np.int64)
    for h in range(n_halves):
        bs[h] = wstart[:, h, :].min(axis=0) // 128
        be[h] = np.maximum(-(-wend[:, h, :].max(axis=0) // 128), bs[h])
    seq_tot = int((be - bs).sum())

    # chunk split per half (shared across cores)
    chunk_sizes = []
    for h in range(n_halves):
        rem, sizes = shared_len[h], []
        while rem > 0:
            L = min(chunk, rem)
            sizes.append(L)
            rem -= L
        chunk_sizes.append(sizes)

    per_core_inputs = []
    lanes = np.arange(128)
    for c in range(n_cores):
        slot_src, slot_dl, slot_win = [], [], []
        for h in range(n_halves):
            sh, dlh, wh = core_streams[c][h]
            ss = np.zeros(shared_len[h], np.int64)
            sd = np.full(shared_len[h], -1, np.int64)
            sw = np.full(shared_len[h], -1, np.int64)
            for g in range(n_grp):
                gs_w, ge_w = g * WGRP, min((g + 1) * WGRP, nwin)
                a = offs_all[c, h, gs_w]
                b = offs_all[c, h, ge_w]
                gb = int(gbase[h, g])
                ss[gb:gb + (b - a)] = sh[a:b]
                sd[gb:gb + (b - a)] = dlh[a:b]
                sw[gb:gb + (b - a)] = wh[a:b]
            slot_src.append(ss)
            slot_dl.append(sd)
            slot_win.append(sw)

        # one-hot pattern per emission seq: for v: for h: for b in [bs, be)
        ohp = np.zeros((128, seq_tot, 128), dtype=ml_dtypes.float8_e4m3)
        seq = 0
        for v in range(nwin):
            for h in range(n_halves):
                for b in range(int(bs[h][v]), int(be[h][v])):
                    sl = slice(b * 128, (b + 1) * 128)
                    msk = slot_win[h][sl] == v
                    ohp[lanes[msk], seq, slot_dl[h][sl][msk]] = 1.0
                    seq += 1
        assert seq == seq_tot

        dv = np.ones((128, nwin), np.float32)
        for wi in range(nwin):
            wl = min(WIN, npc - wi * WIN)
            dv[:wl, wi] = dinv_node[c * npc + wi * WIN:c * npc + wi * WIN + wl]
        core_in = {
            "ohpat": np.ascontiguousarray(ohp.reshape(128, seq_tot * 128)),
            "dinvp": dv,
            "sqdeg": np.ascontiguousarray(
                sqdeg_node[c * npc:(c + 1) * npc].astype(ml_dtypes.bfloat16)
            ).reshape(1, npc),
        }
        # gather indices: wrapped [16, L/16] per chunk, replicated to 128 rows
        for h in range(n_halves):
            stream = slot_src[h].astype(np.int16)
            cols, off = [], 0
            for L in chunk_sizes[h]:
                a = stream[off:off + L].reshape(-1, 16).T
                cols.append(a)
                off += L
            wrapped = np.concatenate(cols, axis=1)
            core_in["idx_h%d" % h] = np.ascontiguousarray(np.tile(wrapped, (8, 1)))
        per_core_inputs.append(core_in)

    meta = dict(n_nodes=n_nodes, n_cores=n_cores, npc=npc, nwin=nwin,
                n_halves=n_halves, split=split, bs=bs, be=be,
                shared_len=shared_len, seq_tot=seq_tot, chunk=chunk,
                chunk_sizes=chunk_sizes, dinv_node=dinv_node)
    return meta, per_core_inputs


# ------------------------------------------------------------- bass program

def build_program(meta):
    import concourse.bacc as bacc
    import concourse.mybir as mybir
    import concourse.tile as tile
    from concourse import library_config

    f32 = mybir.dt.float32
    bf16 = mybir.dt.bfloat16
    fp8 = mybir.dt.float8e4
    i16 = mybir.dt.int16
    AF = mybir.ActivationFunctionType
    OP = mybir.AluOpType

    n_nodes = meta["n_nodes"]
    npc, nwin = meta["npc"], meta["nwin"]
    n_halves, split = meta["n_halves"], meta["split"]
    bs, be = meta["bs"], meta["be"]
    seq_tot = meta["seq_tot"]
    chunk = meta["chunk"]
    chunk_sizes = meta["chunk_sizes"]

    nc = bacc.Bacc("TRN2", num_swdge_queues=N_QUEUES,
                   dynamic_dma_scratch_size=32768)

    x_d = nc.declare_dram_parameter("x", [n_nodes, D], bf16, isOutput=False)
    oh_d = nc.declare_dram_parameter("ohpat", [128, seq_tot * 128], fp8,
                                     isOutput=False)
    dv_d = nc.declare_dram_parameter("dinvp", [128, nwin], f32, isOutput=False)
    sq_d = nc.declare_dram_parameter("sqdeg", [1, npc], bf16, isOutput=False)
    idx_d = [nc.declare_dram_parameter("idx_h%d" % h,
                                       [128, meta["shared_len"][h] // 16],
                                       i16, isOutput=False)
             for h in range(n_halves)]
    w1_d = nc.declare_dram_parameter("W1", [D, D], f32, isOutput=False)
    w2_d = nc.declare_dram_parameter("W2", [D, D], f32, isOutput=False)
    b1_d = nc.declare_dram_parameter("b1", [1, D], f32, isOutput=False)
    b2_d = nc.declare_dram_parameter("b2", [1, D], f32, isOutput=False)
    xo_d = nc.declare_dram_parameter("xown", [npc, D], bf16, isOutput=False)
    id_d = nc.declare_dram_parameter("ident", [128, 128], fp8, isOutput=False)
    out_d = nc.declare_dram_parameter("out", [npc, D], f32, isOutput=True)

    WG = 4  # windows per phase-2 batch (one 512-wide psum bank)

    with tile.TileContext(nc) as tc:
        with (
            tc.tile_pool(name="const", bufs=1) as constp,
            tc.tile_pool(name="xg", bufs=8) as xgp,
            tc.tile_pool(name="ps1", bufs=3, space="PSUM") as ps1,
            tc.tile_pool(name="ps2", bufs=2, space="PSUM") as ps2,
            tc.tile_pool(name="fin", bufs=2) as finp,
        ):
            # Q7 library holding DMAGatherAnt; must precede all gathers
            nc.gpsimd.load_library(library_config.mlp)

            # idx streams first on the sync ring so gathers start ASAP
            idx_all = []
            for h in range(n_halves):
                t = constp.tile([128, meta["shared_len"][h] // 16], i16,
                                tag="idx%d" % h)
                nc.sync.dma_start(t[:], idx_d[h][:])
                idx_all.append(t)

            # xown first (small, needed by every window close), then the
            # big one-hot pieces -- all on the scalar ring, in parallel
            # with the sync ring's preamble
            xow = constp.tile([128, nwin, 128], bf16)
            nfull = npc // 128
            nc.scalar.dma_start(
                xow[:, :nfull, :],
                xo_d[0:nfull * 128, :].rearrange("(w p) f -> p w f", p=128))
            if npc % 128:
                nc.scalar.dma_start(xow[: npc % 128, nfull, :],
                                    xo_d[nfull * 128:npc, :])
            oh_all = constp.tile([128, seq_tot, 128], fp8)
            oh_view = oh_d[:].rearrange("p (b n) -> p b n", n=128)
            step = -(-seq_tot // OH_LOADS)
            for i, a in enumerate(range(0, seq_tot, step)):
                z = min(seq_tot, a + step)
                eng = nc.scalar if i % 2 == 0 else nc.sync
                eng.dma_start(oh_all[:, a:z, :], oh_view[:, a:z, :])

            # --- constants / metadata
            wts = {}
            for nm, src_d in (("w1", w1_d), ("w2", w2_d)):
                raw = constp.tile([128, 128], f32, tag=nm + "raw")
                nc.sync.dma_start(raw[:], src_d[:])
                half = constp.tile([128, 128], bf16, tag=nm + "half")
                nc.scalar.activation(half[:], raw[:], AF.Copy, scale=0.5)
                wts[nm] = half
            bias = {}
            for nm, src_d in (("b1", b1_d), ("b2", b2_d)):
                raw = constp.tile([1, 128], f32, tag=nm + "raw")
                nc.sync.dma_start(raw[:], src_d[:])
                half = constp.tile([1, 128], bf16, tag=nm + "half")
                nc.scalar.activation(half[:], raw[:], AF.Copy, scale=0.5)
                bias[nm] = half
            dinvp = constp.tile([128, nwin], f32)
            nc.sync.dma_start(dinvp[:], dv_d[:])
            sqdeg = constp.tile([1, npc], bf16)
            nc.sync.dma_start(sqdeg[:], sq_d[:])
            ident = constp.tile([128, 128], fp8)
            nc.sync.dma_start(ident[:], id_d[:])

            g_all = constp.tile([128, npc], bf16)

            # per-half stream state: lazy chunk issuing in window order
            class Stream:
                pass

            streams = []
            for h in range(n_halves):
                s = Stream()
                s.h = h
                s.base = x_d[0:split, :] if h == 0 else x_d[split:n_nodes, :]
                s.chunk_bounds = []
                off = 0
                for L in chunk_sizes[h]:
                    s.chunk_bounds.append((off, L))
                    off += L
                s.blk2chunk = np.repeat(
                    np.arange(len(chunk_sizes[h])),
                    [L // 128 for L in chunk_sizes[h]])
                s.tiles = {}
                streams.append(s)

            ci_global = 0

            def ensure_chunk(s, ci):
                nonlocal ci_global
                if ci in s.tiles:
                    return s.tiles[ci]
                off, L = s.chunk_bounds[ci]
                xg = xgp.tile([128, chunk // 128, 128], bf16, tag="xg")
                nc.gpsimd.dma_gather(
                    out_ap=xg[:, : L // 128, :],
                    in_ap=s.base,
                    idxs_ap=idx_all[s.h][:, off // 16:(off + L) // 16],
                    num_idxs=L,
                    num_idxs_reg=L,
                    elem_size=D,
                    single_packet=False,
                    queue_num=ci_global % N_QUEUES,
                )
                ci_global += 1
                # keep two chunks cached: windows whose block range
                # straddles a chunk boundary revisit the previous chunk
                while len(s.tiles) >= 2:
                    del s.tiles[min(s.tiles)]
                s.tiles[ci] = xg
                return xg

            # --- output stage: every WG windows, two weight matmuls +
            # per-window dinv-scaled relu + combine; overlaps aggregation
            def emit_phase2(wlo, whi):
                nwg = whi - wlo + 1
                wls = [min(WIN, npc - w * WIN) for w in range(wlo, whi + 1)]
                pps = {}
                for nm_w, nm_b in (("w1", "b1"), ("w2", "b2")):
                    pp = ps2.tile([128, WG * 128], f32, tag="pp")
                    for j, w in enumerate(range(wlo, whi + 1)):
                        wl = wls[j]
                        sl = pp[:wl, j * 128:(j + 1) * 128]
                        nc.tensor.matmul(sl, g_all[:, w * WIN:w * WIN + wl],
                                         wts[nm_w][:], start=True, stop=False)
                        nc.tensor.matmul(sl, sqdeg[:, w * WIN:w * WIN + wl],
                                         bias[nm_b][:], start=False, stop=True)
                    o = finp.tile([128, WG, 128], f32, tag="o" + nm_w)
                    for j, w in enumerate(range(wlo, whi + 1)):
                        nc.scalar.activation(
                            o[:wls[j], j, :],
                            pp[:wls[j], j * 128:(j + 1) * 128], AF.Relu,
                            scale=dinvp[:wls[j], w:w + 1])
                    pps[nm_w] = o
                ot = finp.tile([128, WG, 128], f32, tag="ot")
                otf = ot[:].rearrange("p c n -> p (c n)")
                o1f = pps["w1"][:].rearrange("p c n -> p (c n)")
                o2f = pps["w2"][:].rearrange("p c n -> p (c n)")
                rows = min(wls)
                if rows == 128:
                    nc.vector.tensor_tensor(otf[:, :nwg * 128],
                                            o1f[:, :nwg * 128],
                                            o2f[:, :nwg * 128], op=OP.add)
                else:
                    for j in range(nwg):
                        cs = slice(j * 128, j * 128 + 128)
                        nc.vector.tensor_tensor(otf[:wls[j], cs],
                                                o1f[:wls[j], cs],
                                                o2f[:wls[j], cs], op=OP.add)
                for j, w in enumerate(range(wlo, whi + 1)):
                    nc.sync.dma_start(out_d[w * WIN:w * WIN + wls[j], :],
                                      ot[:wls[j], j, :])

            seq = 0
            for w in range(nwin):
                wlen = min(WIN, npc - w * WIN)
                pw = ps1.tile([128, 128], f32, tag="pw")
                n_tot = 1 + sum(int(be[s.h][w] - bs[s.h][w]) for s in streams)
                k = 0
                for s in streams:
                    for b in range(int(bs[s.h][w]), int(be[s.h][w])):
                        ci = int(s.blk2chunk[b])
                        xg = ensure_chunk(s, ci)
                        bl = (b * 128 - s.chunk_bounds[ci][0]) // 128
                        nc.tensor.matmul(
                            pw[:, :wlen],
                            xg[:, bl, :],
                            oh_all[:, seq, :wlen],
                            start=(k == 0),
                            stop=False,
                        )
                        seq += 1
                        k += 1
                # self-loop last: pw += xown_w^T @ I  (exact fp8 identity);
                # placed at the close so edge matmuls never wait on xow
                nc.tensor.matmul(pw[:, :wlen], xow[:wlen, w, :],
                                 ident[:wlen, :wlen],
                                 start=(k == 0), stop=True)
                nc.scalar.activation(g_all[:, w * WIN:w * WIN + wlen],
                                     pw[:, :wlen], AF.Copy)
                if w % WG == WG - 1 or w == nwin - 1:
                    emit_phase2(w - (w % WG), w)
            assert seq == seq_tot

    nc.compile()
    return nc


def make_core_inputs(meta, per_core_inputs, x, W1, b1, W2, b2):
    """Full in_maps for run_bass_kernel_spmd (adds shared tensors)."""
    import ml_dtypes
    xf = np.asarray(x, np.float32)
    dinv = meta["dinv_node"]
    xp = (xf * dinv[:, None]).astype(ml_dtypes.bfloat16)
    npc = meta["npc"]
    shared = {
        "x": np.ascontiguousarray(xp),
        "W1": np.ascontiguousarray(np.asarray(W1, np.float32)),
        "W2": np.ascontiguousarray(np.asarray(W2, np.float32)),
        "b1": np.asarray(b1, np.float32).reshape(1, D),
        "b2": np.asarray(b2, np.float32).reshape(1, D),
        "ident": np.ascontiguousarray(
            np.eye(128, dtype=ml_dtypes.float8_e4m3)),
    }
    maps = []
    for c, ci in enumerate(per_core_inputs):
        ci = dict(ci)
        ci["xown"] = np.ascontiguousarray(xp[c * npc:(c + 1) * npc])
        maps.append(dict(shared, **ci))
    return maps


# ------------------------------------------------------------------- kernel

def kernel(x, edge_index, W1, b1, W2, b2, _trace=False):
    from concourse.bass_utils import run_bass_kernel_spmd

    x = np.asarray(x)
    n_nodes = x.shape[0]
    meta, pci = host_prep(edge_index, n_nodes, N_CORES)
    nc = build_program(meta)
    in_maps = make_core_inputs(meta, pci, x, W1, b1, W2, b2)
    res = run_bass_kernel_spmd(nc, in_maps, list(range(N_CORES)),
                               trace=_trace)
    out = np.concatenate([res.results[c]["out"] for c in range(N_CORES)],
                         axis=0)
    if _trace:
        return out, res
    return out
